# revision 1
# baseline (speedup 1.0000x reference)
"""GATv2 layer on 8 Trainium2 NeuronCores (Bass/Tile).

Strategy: sort edges by dst node on host; core k owns dst nodes
[2500k, 2500(k+1)) so segment softmax + aggregation are core-local (no
collectives). Edges are bucketed into 20 blocks of 128 dst nodes per core and
padded per block to a uniform tile count so one SPMD program serves all cores.

Per core on device:
  phase 1: node projections  hsv = [x@W1.T | 0.2*att-dot | x@W.T] (bf16, DRAM)
           hd  = [x@W2.T | 0.2*att-dot] for local dst nodes (bf16, DRAM)
  phase 2: per 128-edge tile: indirect-gather hsv[src], hd[dst];
           z = hs + hd (also alpha_s + alpha_d for the linear logit term);
           zT via PE transpose; r = Relu(zT) (ACT);
           logits = 0.8*att.T @ r  (+ linear term)  [PE, att folded into rhs]
           exp (ACT); weighted = exp * vals (DVE);
           segment-sum via one-hot matmul accumulated in PSUM per block;
           normalize by denominator + bias, DMA out.

Perf note (cost model, InstructionCostModel): ~634us/core. Bottleneck is the
360 indirect_dma_start gathers: SWDGE_FIXED_OVERHEAD_NS=994 is charged per
instruction while holding the gpsimd engine (~373us serial). Next step:
replace the 8 per-super indirect DMAs with ONE gpsimd.dma_gather
(num_idxs=1024, int16 indices wrapped [16, 64], elem_size padded to a
256-byte multiple, i.e. hsv row 520->640 bf16) -> one 994ns fixed cost per
super instead of eight, est. ~310us saved. Optionally prepare_only=True +
trigger_dma to pre-generate descriptors during the projection phase.
"""
import os
import sys

sys.path.insert(0, '/opt/trn_rl_repo')

import numpy as np
import ml_dtypes

N = 20000
IN_F = 128
HEADS = 8
OUT_F = 32
HF = 256          # HEADS * OUT_F
NEG = 0.2
CORES = 8
NPC = 2500        # dst nodes per core
BLOCKS = 20       # 128-node blocks per core (2560 >= 2500)
NLOC = BLOCKS * 128
NT_GLOB = 157     # ceil(20000/128)
NPADG = NT_GLOB * 128

bf16 = ml_dtypes.bfloat16

_CACHE = {}
LAST_EXEC_NS = None


def _build(T_blk):
    import concourse.bass as bass
    from concourse import mybir, bacc
    from concourse.tile import TileContext

    f32 = mybir.dt.float32
    b16 = mybir.dt.bfloat16
    i32 = mybir.dt.int32
    AF = mybir.ActivationFunctionType
    ALU = mybir.AluOpType

    n_tiles = BLOCKS * T_blk
    n_super = n_tiles // 8

    nc = bacc.Bacc("TRN2", target_bir_lowering=False, debug=False,
                   num_devices=CORES)
    xT = nc.dram_tensor("xt", [128, NPADG], f32, kind="ExternalInput")
    xTl = nc.dram_tensor("xtl", [128, NLOC], f32, kind="ExternalInput")
    wcat = nc.dram_tensor("wcat", [128, 520], f32, kind="ExternalInput")
    w2cat = nc.dram_tensor("w2cat", [128, 264], f32, kind="ExternalInput")
    attb = nc.dram_tensor("attblk", [128, 16], b16, kind="ExternalInput")
    ident = nc.dram_tensor("ident", [128, 128], b16, kind="ExternalInput")
    iota = nc.dram_tensor("iota", [128, 1024], f32, kind="ExternalInput")
    biasr = nc.dram_tensor("biasr", [128, 256], f32, kind="ExternalInput")
    srcc = nc.dram_tensor("srcc", [128, n_tiles], i32, kind="ExternalInput")
    qdstc = nc.dram_tensor("qdstc", [128, n_tiles], f32, kind="ExternalInput")
    qdT = nc.dram_tensor("qdt", [128, n_tiles * 128], f32, kind="ExternalInput")
    iotaP = nc.dram_tensor("iotap", [128, 1024], f32, kind="ExternalInput")
    outt = nc.dram_tensor("out", [NLOC, 256], f32, kind="ExternalOutput")

    hsv_d = nc.dram_tensor("hsvd", [NPADG, 520], b16, kind="Internal")

    with TileContext(nc) as tc:
        with tc.tile_pool(name="const", bufs=1) as cp:
            wcat_sb = cp.tile([128, 520], f32)
            nc.sync.dma_start(wcat_sb[:], wcat[:])
            w2_sb = cp.tile([128, 264], f32)
            nc.sync.dma_start(w2_sb[:], w2cat[:])
            attb_sb = cp.tile([128, 16], b16)
            nc.sync.dma_start(attb_sb[:], attb[:])
            id_sb = cp.tile([128, 128], b16)
            nc.sync.dma_start(id_sb[:], ident[:])
            iota_sb = cp.tile([128, 1024], f32)
            nc.sync.dma_start(iota_sb[:], iota[:])
            bias_sb = cp.tile([128, 256], f32)
            nc.sync.dma_start(bias_sb[:], biasr[:])
            src_sb = cp.tile([128, n_tiles], i32)
            nc.sync.dma_start(src_sb[:], srcc[:])
            iop_sb = cp.tile([128, 1024], f32)
            nc.sync.dma_start(iop_sb[:], iotaP[:])
            qd_sb = cp.tile([128, n_tiles], f32)
            nc.sync.dma_start(qd_sb[:], qdstc[:])

            # ---------------- phase 1: projections ----------------
            with tc.tile_pool(name="proj", bufs=4) as pp, \
                 tc.tile_pool(name="pps", bufs=4, space="PSUM") as pps:
                for j in range(NT_GLOB):
                    xt_t = pp.tile([128, 128], f32)
                    nc.sync.dma_start(xt_t[:], xT[:, j * 128:(j + 1) * 128])
                    psA = pps.tile([128, 264], f32)
                    nc.tensor.matmul(psA[:], lhsT=xt_t[:],
                                     rhs=wcat_sb[:, 0:264],
                                     start=True, stop=True)
                    psB = pps.tile([128, 256], f32)
                    nc.tensor.matmul(psB[:], lhsT=xt_t[:],
                                     rhs=wcat_sb[:, 264:520],
                                     start=True, stop=True)
                    hv = pp.tile([128, 520], b16)
                    nc.scalar.copy(hv[:, 0:264], psA[:])
                    nc.vector.tensor_copy(hv[:, 264:520], psB[:])
                    nc.sync.dma_start(hsv_d[j * 128:(j + 1) * 128, :], hv[:])
                hd_tiles = []
                for j in range(BLOCKS):
                    xt_t = pp.tile([128, 128], f32)
                    nc.sync.dma_start(xt_t[:], xTl[:, j * 128:(j + 1) * 128])
                    psA = pps.tile([128, 264], f32)
                    nc.tensor.matmul(psA[:], lhsT=xt_t[:], rhs=w2_sb[:],
                                     start=True, stop=True)
                    hv2 = cp.tile([128, 264], b16, tag=f"hd{j}")
                    nc.vector.tensor_copy(hv2[:], psA[:])
                    hd_tiles.append(hv2)

            # ---------------- phase 2: edges ----------------
            with tc.tile_pool(name="edge", bufs=3) as ep, \
                 tc.tile_pool(name="pair", bufs=3) as rp, \
                 tc.tile_pool(name="zps", bufs=3, space="PSUM") as zps, \
                 tc.tile_pool(name="lps", bufs=3, space="PSUM") as lps, \
                 tc.tile_pool(name="aps", bufs=2, space="PSUM") as aps, \
                 tc.tile_pool(name="np_", bufs=2) as npl:
                aggp = None
                for s in range(n_super):
                    hsv_g = ep.tile([128, 8 * 520], b16, tag="hsvg")
                    for t in range(8):
                        g = s * 8 + t
                        nc.gpsimd.indirect_dma_start(
                            out=hsv_g[:, t * 520:(t + 1) * 520],
                            out_offset=None, in_=hsv_d[:],
                            in_offset=bass.IndirectOffsetOnAxis(
                                ap=src_sb[:, g:g + 1], axis=0))
                    qdt_sb = ep.tile([128, 1024], f32, tag="qdt")
                    nc.sync.dma_start(
                        qdt_sb[:], qdT[:, s * 1024:(s + 1) * 1024])
                    ohT = ep.tile([128, 8 * 128], b16, tag="ohT")
                    nc.vector.tensor_tensor(
                        out=ohT[:], in0=qdt_sb[:], in1=iop_sb[:],
                        op=ALU.is_equal)
                    # one-hot [128, 8, 128]
                    oh = ep.tile([128, 8 * 128], b16, tag="oh")
                    nc.vector.tensor_tensor(
                        out=oh[:].rearrange("p (t c) -> p t c", t=8),
                        in0=iota_sb[:].rearrange("p (t c) -> p t c", t=8),
                        in1=qd_sb[:, s * 8:(s + 1) * 8].unsqueeze(2)
                            .broadcast_to([128, 8, 128]),
                        op=ALU.is_equal)
                    wv = ep.tile([128, 8 * 264], b16, tag="wv")
                    for q in range(4):   # pairs within super
                        t0 = 2 * q
                        zTp = zps.tile([128, 512], f32)
                        blk = (s * 8 + t0) // T_blk
                        hdt = hd_tiles[blk]
                        for sl in range(2):
                            t = t0 + sl
                            base = t * 520
                            for hf in range(2):
                                dst_sl = zTp[:, sl * 256 + hf * 128:
                                             sl * 256 + hf * 128 + 128]
                                nc.tensor.matmul(
                                    dst_sl,
                                    lhsT=hsv_g[:, base + hf * 128:
                                               base + hf * 128 + 128],
                                    rhs=id_sb[:], start=True, stop=False)
                                nc.tensor.matmul(
                                    dst_sl,
                                    lhsT=hdt[:, hf * 128:hf * 128 + 128],
                                    rhs=ohT[:, t * 128:(t + 1) * 128],
                                    start=False, stop=True)
                        rT = rp.tile([128, 512], b16, tag="rT")
                        nc.scalar.activation(rT[:], zTp[:], AF.Relu)
                        lgt = lps.tile([128, 16], f32)
                        for sl in range(2):
                            t = t0 + sl
                            nc.tensor.matmul(
                                lgt[:, sl * 8:(sl + 1) * 8],
                                lhsT=rT[:, sl * 256:sl * 256 + 128],
                                rhs=attb_sb[:, 0:8], start=True, stop=False)
                            nc.tensor.matmul(
                                lgt[:, sl * 8:(sl + 1) * 8],
                                lhsT=rT[:, sl * 256 + 128:sl * 256 + 256],
                                rhs=attb_sb[:, 8:16], start=False, stop=False)
                            nc.tensor.matmul(
                                lgt[:, sl * 8:(sl + 1) * 8],
                                lhsT=id_sb[:],
                                rhs=hsv_g[:, t * 520 + 256:t * 520 + 264],
                                start=False, stop=False)
                            nc.tensor.matmul(
                                lgt[:, sl * 8:(sl + 1) * 8],
                                lhsT=ohT[:, t * 128:(t + 1) * 128],
                                rhs=hdt[:, 256:264],
                                start=False, stop=True)
                        # exp -> wv[:, {t0,t0+1}, 256:264]
                        nc.scalar.activation(
                            wv[:].rearrange("p (t c) -> p t c", t=8)
                                [:, t0:t0 + 2, 256:264],
                            lgt[:].rearrange("p (a b) -> p a b", a=2),
                            AF.Exp)
                    # weighted = vals * exp  [128, 8, 8, 32]
                    nc.vector.tensor_tensor(
                        out=wv[:].rearrange("p (t c) -> p t c", t=8)
                            [:, :, 0:256].rearrange(
                                "p t (h f) -> p t h f", h=8),
                        in0=hsv_g[:].rearrange("p (t c) -> p t c", t=8)
                            [:, :, 264:520].rearrange(
                                "p t (h f) -> p t h f", h=8),
                        in1=wv[:].rearrange("p (t c) -> p t c", t=8)
                            [:, :, 256:264].unsqueeze(3)
                            .broadcast_to([128, 8, 8, 32]),
                        op=ALU.mult)
                    for t in range(8):
                        g = s * 8 + t
                        if g % T_blk == 0:
                            aggp = aps.tile([128, 264], f32, tag="agg")
                        nc.tensor.matmul(
                            aggp[:],
                            lhsT=oh[:, t * 128:(t + 1) * 128],
                            rhs=wv[:, t * 264:(t + 1) * 264],
                            start=(g % T_blk == 0),
                            stop=(g % T_blk == T_blk - 1))
                        if g % T_blk == T_blk - 1:
                            b = g // T_blk
                            dn = npl.tile([128, 8], f32, tag="dn")
                            nc.vector.tensor_scalar(
                                out=dn[:], in0=aggp[:, 256:264],
                                scalar1=1e-12, scalar2=None, op0=ALU.max)
                            rec = npl.tile([128, 8], f32, tag="rec")
                            nc.vector.reciprocal(rec[:], dn[:])
                            osb = npl.tile([128, 256], f32, tag="osb")
                            nc.vector.tensor_tensor(
                                out=osb[:].rearrange("p (h f) -> p h f", h=8),
                                in0=aggp[:, 0:256].rearrange(
                                    "p (h f) -> p h f", h=8),
                                in1=rec[:].unsqueeze(2)
                                    .broadcast_to([128, 8, 32]),
                                op=ALU.mult)
                            nc.vector.tensor_add(osb[:], osb[:], bias_sb[:])
                            nc.sync.dma_start(
                                outt[b * 128:(b + 1) * 128, :], osb[:])
    nc.compile()
    return nc


def _prep(x, edge_index, W, W1, W2, att, bias):
    x = np.asarray(x, np.float32)
    ei = np.asarray(edge_index)
    W = np.asarray(W, np.float32)
    W1 = np.asarray(W1, np.float32)
    W2 = np.asarray(W2, np.float32)
    att = np.asarray(att, np.float32)
    bias = np.asarray(bias, np.float32)

    src = ei[0].astype(np.int64)
    dst = ei[1].astype(np.int64)
    perm = np.argsort(dst, kind='stable')
    src_s = src[perm].astype(np.int32)
    dst_s = dst[perm].astype(np.int32)

    # per (core, block) counts
    blk_of = dst_s // 128          # global 128-blocks: 157 of them; but per
    core_of = dst_s // NPC
    # local block index within core
    lblk = (dst_s - core_of * NPC) // 128
    cnt = np.zeros((CORES, BLOCKS), np.int64)
    np.add.at(cnt, (core_of, lblk), 1)
    T_blk = int(np.ceil(cnt.max() / 128))
    if T_blk % 2:
        T_blk += 1
    n_tiles = BLOCKS * T_blk

    # padded per-core edge arrays
    srcc = np.zeros((CORES, n_tiles * 128), np.int32)
    ldst = np.zeros((CORES, n_tiles * 128), np.int32)
    qdst = np.full((CORES, n_tiles * 128), -1.0, np.float32)
    order = np.lexsort((np.arange(len(dst_s)), lblk, core_of))  # stable
    # edges already sorted by dst -> core_of/lblk sorted; just use ranges
    for k in range(CORES):
        for b in range(BLOCKS):
            c = cnt[k, b]
            if c == 0:
                continue
            # contiguous range in sorted arrays
            # start index: edges with (core<k) + (core==k, blk<b)
            lo = np.searchsorted(dst_s, k * NPC + b * 128)
            hi = lo + c
            base = b * T_blk * 128
            srcc[k, base:base + c] = src_s[lo:hi]
            ld = dst_s[lo:hi] - k * NPC
            ldst[k, base:base + c] = ld
            qdst[k, base:base + c] = (ld - b * 128).astype(np.float32)

    # constants
    was02 = NEG * np.einsum('ihf,hf->ih',
                            W1.T.reshape(IN_F, HEADS, OUT_F), att[0])
    wad02 = NEG * np.einsum('ihf,hf->ih',
                            W2.T.reshape(IN_F, HEADS, OUT_F), att[0])
    wcat = np.concatenate([W1.T, was02, W.T], axis=1).astype(np.float32)
    w2cat = np.concatenate([W2.T, wad02], axis=1).astype(np.float32)

    attb = np.zeros((128, 16), np.float32)
    for p in range(128):
        attb[p, p // 32] = (1.0 - NEG) * att[0, p // 32, p % 32]
        attb[p, 8 + 4 + p // 32] = (1.0 - NEG) * att[0, 4 + p // 32, p % 32]
    attb = attb.astype(bf16)

    x_pad = np.zeros((NPADG, IN_F), np.float32)
    x_pad[:N] = x
    xT = np.ascontiguousarray(x_pad.T)
    iota = np.tile(np.arange(128, dtype=np.float32), (128, 8))
    iotap = np.ascontiguousarray(
        np.tile(np.arange(128, dtype=np.float32)[:, None], (1, 1024)))
    biasr = np.tile(bias[None, :], (128, 1)).astype(np.float32)
    ident = np.eye(128, dtype=np.float32).astype(bf16)

    in_maps = []
    for k in range(CORES):
        xl = np.ascontiguousarray(
            x_pad[k * NPC:k * NPC + NLOC].T).astype(np.float32)
        in_maps.append({
            "xt": xT, "xtl": xl, "wcat": wcat, "w2cat": w2cat,
            "attblk": attb, "ident": ident, "iota": iota, "biasr": biasr,
            "srcc": np.ascontiguousarray(
                srcc[k].reshape(n_tiles, 128).T),
            "qdstc": np.ascontiguousarray(
                qdst[k].reshape(n_tiles, 128).T),
            "qdt": np.ascontiguousarray(
                np.tile(qdst[k][None, :], (128, 1))),
            "iotap": iotap,
        })
    return T_blk, in_maps


def kernel(x, edge_index, W, W1, W2, att, bias):
    global LAST_EXEC_NS
    from concourse import bass_utils

    T_blk, in_maps = _prep(x, edge_index, W, W1, W2, att, bias)
    if T_blk not in _CACHE:
        _CACHE[T_blk] = _build(T_blk)
    nc = _CACHE[T_blk]

    trace = bool(int(os.environ.get("GAT_TRACE", "0")))
    res = bass_utils.run_bass_kernel_spmd(
        nc, in_maps, core_ids=list(range(CORES)), trace=trace)
    LAST_EXEC_NS = res.exec_time_ns

    out = np.empty((N, HF), np.float32)
    for k in range(CORES):
        out[k * NPC:(k + 1) * NPC] = res.results[k]["out"][:NPC]
    return out



# revision 7
# speedup vs baseline: 1.7041x; 1.7041x over previous
"""GATv2 layer on 8 Trainium2 NeuronCores (Bass/Tile) — v3.

Strategy: sort edges by dst on host; core k owns dst nodes [2500k, 2500(k+1))
so softmax + aggregation are core-local. Edges bucketed into 20 blocks of 128
dst nodes per core, padded per block to T_blk tiles of 128 edges (uniform so
one SPMD program serves all cores). 8 tiles = 1 super (1024 edges).

Math notes:
 - logit l_e = att·leaky_relu(hs[src]+hd[dst], 0.2). The ACT engine's Prelu
   (parametric_relu, alpha=0.2) IS leaky relu, so no Relu/linear split.
 - Any per-dst-node additive logit term cancels in the segment softmax
   (exp factor cancels numerator/denominator), so no seg-max subtraction.

Device phases (per core):
  phase 1 (bf16): node table hsv_d[n] = [x@W1.T | x@W.T] (20096 x 512 bf16,
    1KB rows, staged 8 node-tiles per DMA); hd = x@W2.T for the 2560 local
    dst nodes (SBUF-resident).
  phase 2, per super (1024 edges):
    - ONE dma_gather (1024 idx, 1KB rows): hsv_g [128, 8, 512] edge-major.
    - qdT = partition_broadcast(dst-id row) [Pool]; ohT = is_equal(qdT,
      iota_p) [DVE, node-on-partition one-hot]; oh = is_equal(qd, iota_f)
      [Pool, edge-major one-hot].
    - per pair (2 tiles): zT PSUM = PE-transpose(hs slices) + hd[blk]@ohT;
      lr = Prelu(zT) [ACT]; lgt[128,16] = lr@attb (4 small matmuls, full
      PE-array util); exp [ACT]; wv = v*exp [DVE, exp appended for denoms];
      segment-sum via oh-matmul accumulated in PSUM per dst block;
      at block end: normalize by denominator + bias, DMA out.

Cost-model (TimelineSim InstructionCostModel): ~300us/core vs 634us baseline
(Pool-serialized indirect DMAs at 994ns fixed each). Walls: DMA ~217us,
DVE ~195, Pool ~195, ACT ~180, PE ~165.
"""
import os
import sys

sys.path.insert(0, '/opt/trn_rl_repo')

import numpy as np
import ml_dtypes

N = 20000
IN_F = 128
HEADS = 8
OUT_F = 32
HF = 256          # HEADS * OUT_F
NEG = 0.2
CORES = 8
NPC = 2500        # dst nodes per core
BLOCKS = 20       # 128-node blocks per core
NLOC = BLOCKS * 128
NT_GLOB = 157     # ceil(20000/128)
NPADG = NT_GLOB * 128

bf16 = ml_dtypes.bfloat16

_CACHE = {}
LAST_EXEC_NS = None


def _build(T_blk):
    import concourse.bass as bass
    from concourse import mybir, bacc
    from concourse.tile import TileContext

    f32 = mybir.dt.float32
    b16 = mybir.dt.bfloat16
    i16 = mybir.dt.int16
    AF = mybir.ActivationFunctionType
    ALU = mybir.AluOpType

    n_tiles = BLOCKS * T_blk
    n_super = n_tiles // 8

    nc = bacc.Bacc("TRN2", target_bir_lowering=False, debug=False,
                   num_devices=CORES)
    xT = nc.dram_tensor("xt", [128, NPADG], b16, kind="ExternalInput")
    xTl = nc.dram_tensor("xtl", [128, NLOC], b16, kind="ExternalInput")
    wcat = nc.dram_tensor("wcat", [128, 512], b16, kind="ExternalInput")
    w2T = nc.dram_tensor("w2t", [128, 256], b16, kind="ExternalInput")
    attb = nc.dram_tensor("attb", [128, 16], b16, kind="ExternalInput")
    ident = nc.dram_tensor("ident", [128, 128], b16, kind="ExternalInput")
    iop = nc.dram_tensor("iop", [128, 1024], b16, kind="ExternalInput")
    iotaf = nc.dram_tensor("iotaf", [128, 1024], b16, kind="ExternalInput")
    qdsb = nc.dram_tensor("qdsb", [128, n_tiles], b16, kind="ExternalInput")
    qdrow = nc.dram_tensor("qdrow", [1, n_tiles * 128], b16,
                           kind="ExternalInput")
    srcw = nc.dram_tensor("srcw", [128, n_super * 64], i16,
                          kind="ExternalInput")
    biasr = nc.dram_tensor("biasr", [128, 256], f32, kind="ExternalInput")
    outt = nc.dram_tensor("out", [NLOC, 256], f32, kind="ExternalOutput")

    hsv_d = nc.dram_tensor("hsvd", [NPADG, 512], b16, kind="Internal")

    with TileContext(nc) as tc:
        with tc.tile_pool(name="const", bufs=1) as cp:
            wcat_sb = cp.tile([128, 512], b16)
            nc.sync.dma_start(wcat_sb[:], wcat[:])
            w2_sb = cp.tile([128, 256], b16)
            nc.sync.dma_start(w2_sb[:], w2T[:])
            attb_sb = cp.tile([128, 16], b16)
            nc.sync.dma_start(attb_sb[:], attb[:])
            id_sb = cp.tile([128, 128], b16)
            nc.sync.dma_start(id_sb[:], ident[:])
            iop_sb = cp.tile([128, 1024], b16)
            nc.sync.dma_start(iop_sb[:], iop[:])
            iof_sb = cp.tile([128, 1024], b16)
            nc.sync.dma_start(iof_sb[:], iotaf[:])
            qd_sb = cp.tile([128, n_tiles], b16)
            nc.sync.dma_start(qd_sb[:], qdsb[:])
            src_sb = cp.tile([128, n_super * 64], i16)
            nc.sync.dma_start(src_sb[:], srcw[:])
            bias_sb = cp.tile([128, 256], f32)
            nc.sync.dma_start(bias_sb[:], biasr[:])

            # ---------------- phase 1: projections (bf16) ----------------
            NJC = 20    # node tiles per x chunk
            NST = 8     # node tiles per staging/write batch
            with tc.tile_pool(name="proj", bufs=2) as pp, \
                 tc.tile_pool(name="pst", bufs=2) as pstp, \
                 tc.tile_pool(name="pps", bufs=4, space="PSUM") as pps:
                cast_rr = 0
                stage = None
                for j in range(NT_GLOB):
                    if j % NJC == 0:
                        xc = pp.tile([128, 2560], b16, tag="xc")
                        hi = min((j + NJC) * 128, NPADG)
                        nc.sync.dma_start(xc[:, 0:hi - j * 128],
                                          xT[:, j * 128:hi])
                    if j % NST == 0:
                        stage = pstp.tile([128, NST * 512], b16, tag="st")
                    xcol = (j % NJC) * 128
                    ps = pps.tile([128, 512], f32)
                    nc.tensor.matmul(ps[:], lhsT=xc[:, xcol:xcol + 128],
                                     rhs=wcat_sb[:], start=True, stop=True)
                    dstsl = stage[:, (j % NST) * 512:(j % NST) * 512 + 512]
                    if cast_rr == 0:
                        nc.scalar.copy(dstsl, ps[:])
                    else:
                        nc.vector.tensor_copy(dstsl, ps[:])
                    cast_rr = (cast_rr + 1) % 2
                    if j % NST == NST - 1 or j == NT_GLOB - 1:
                        j0 = (j // NST) * NST
                        nt = j - j0 + 1
                        nc.sync.dma_start(
                            hsv_d[j0 * 128:(j0 + nt) * 128, :].rearrange(
                                "(t p) f -> p t f", p=128),
                            stage[:, 0:nt * 512].rearrange(
                                "p (t f) -> p t f", f=512))
                hd_tiles = []
                xl = pp.tile([128, NLOC], b16, tag="xl")
                nc.sync.dma_start(xl[:], xTl[:])
                for b in range(BLOCKS):
                    ps2 = pps.tile([128, 256], f32)
                    nc.tensor.matmul(ps2[:], lhsT=xl[:, b * 128:b * 128 + 128],
                                     rhs=w2_sb[:], start=True, stop=True)
                    hdt = cp.tile([128, 256], b16, tag=f"hd{b}")
                    if b % 2 == 0:
                        nc.scalar.copy(hdt[:], ps2[:])
                    else:
                        nc.vector.tensor_copy(hdt[:], ps2[:])
                    hd_tiles.append(hdt)

            # ---------------- phase 2: edges ----------------
            with tc.tile_pool(name="edge", bufs=2) as ep, \
                 tc.tile_pool(name="pair", bufs=3) as rp, \
                 tc.tile_pool(name="zps", bufs=3, space="PSUM") as zps, \
                 tc.tile_pool(name="lps", bufs=2, space="PSUM") as lps, \
                 tc.tile_pool(name="aps", bufs=2, space="PSUM") as aps, \
                 tc.tile_pool(name="np_", bufs=2) as npl:
                aggp = None
                for s in range(n_super):
                    idsl = src_sb[:, s * 64:(s + 1) * 64]
                    hsv_g = ep.tile([128, 8 * 512], b16, tag="hsvg")
                    nc.gpsimd.dma_gather(
                        hsv_g[:].rearrange("p (t f) -> p t f", f=512),
                        hsv_d[:], idsl, 1024, 1024, 512,
                        transpose=False)
                    qstage = ep.tile([128, 1024], b16, tag="qst")
                    nc.sync.dma_start(qstage[0:1, :],
                                      qdrow[:, s * 1024:(s + 1) * 1024])
                    qdT = ep.tile([128, 1024], b16, tag="qdT")
                    nc.gpsimd.partition_broadcast(qdT[:], qstage[0:1, :])
                    ohT = ep.tile([128, 1024], b16, tag="ohT")
                    nc.vector.tensor_tensor(out=ohT[:], in0=qdT[:],
                                            in1=iop_sb[:], op=ALU.is_equal)
                    oh = ep.tile([128, 1024], b16, tag="oh")
                    nc.vector.tensor_tensor(
                        out=oh[:].rearrange("p (t c) -> p t c", t=8),
                        in0=qd_sb[:, s * 8:(s + 1) * 8].unsqueeze(2)
                            .broadcast_to([128, 8, 128]),
                        in1=iof_sb[:].rearrange("p (t c) -> p t c", t=8),
                        op=ALU.is_equal)
                    for q in range(4):   # pairs within super
                        t0 = s * 8 + 2 * q
                        blk = t0 // T_blk
                        hdt = hd_tiles[blk]
                        # zT [128, (hf, sl, e)] in PSUM; per 128-col region:
                        # transpose(hs) opens the accum group, hd one-hot
                        # closes it (groups strictly sequential per region)
                        psZ = zps.tile([128, 512], f32)
                        for hf in range(2):
                            for sl in range(2):
                                reg = psZ[:, hf * 256 + sl * 128:
                                          hf * 256 + sl * 128 + 128]
                                nc.tensor.matmul(
                                    reg,
                                    lhsT=hsv_g[:, (2 * q + sl) * 512 +
                                               hf * 128:
                                               (2 * q + sl) * 512 +
                                               hf * 128 + 128],
                                    rhs=id_sb[:], start=True, stop=False)
                                nc.tensor.matmul(
                                    reg,
                                    lhsT=hdt[:, hf * 128:hf * 128 + 128],
                                    rhs=ohT[:, (2 * q + sl) * 128:
                                            (2 * q + sl) * 128 + 128],
                                    start=False, stop=True)
                        lr = rp.tile([128, 512], b16, tag="lr")
                        nc.scalar.activation(lr[:], psZ[:], AF.Prelu,
                                             alpha=NEG)
                        lgt = lps.tile([128, 16], f32)
                        for sl in range(2):
                            for hf in range(2):
                                nc.tensor.matmul(
                                    lgt[:, sl * 8:sl * 8 + 8],
                                    lhsT=lr[:, hf * 256 + sl * 128:
                                            hf * 256 + sl * 128 + 128],
                                    rhs=attb_sb[:, hf * 8:hf * 8 + 8],
                                    start=(hf == 0), stop=(hf == 1))
                        wvp = rp.tile([128, 528], b16, tag="wvp")
                        nc.scalar.activation(
                            wvp[:].rearrange("p (t c) -> p t c", t=2)
                                [:, :, 256:264],
                            lgt[:].rearrange("p (a b) -> p a b", a=2),
                            AF.Exp)
                        nc.vector.tensor_tensor(
                            out=wvp[:].rearrange("p (t c) -> p t c", t=2)
                                [:, :, 0:256].rearrange(
                                    "p t (h f) -> p t h f", h=8),
                            in0=hsv_g[:].rearrange("p (t f) -> p t f", f=512)
                                [:, 2 * q:2 * q + 2, 256:512].rearrange(
                                    "p t (h f) -> p t h f", h=8),
                            in1=wvp[:].rearrange("p (t c) -> p t c", t=2)
                                [:, :, 256:264].unsqueeze(3)
                                .broadcast_to([128, 2, 8, 32]),
                            op=ALU.mult)
                        for sl in range(2):
                            t = t0 + sl
                            tb = t % T_blk
                            if tb == 0:
                                aggp = aps.tile([128, 264], f32, tag="agg")
                            nc.tensor.matmul(
                                aggp[:],
                                lhsT=oh[:, (2 * q + sl) * 128:
                                        (2 * q + sl) * 128 + 128],
                                rhs=wvp[:, sl * 264:sl * 264 + 264],
                                start=(tb == 0), stop=(tb == T_blk - 1))
                            if tb == T_blk - 1:
                                b = t // T_blk
                                dn = npl.tile([128, 8], f32, tag="dn")
                                nc.vector.tensor_scalar(
                                    out=dn[:], in0=aggp[:, 256:264],
                                    scalar1=1e-12, scalar2=None, op0=ALU.max)
                                rec = npl.tile([128, 8], f32, tag="rec")
                                nc.vector.reciprocal(rec[:], dn[:])
                                osb = npl.tile([128, 256], f32, tag="osb")
                                nc.vector.tensor_tensor(
                                    out=osb[:].rearrange(
                                        "p (h f) -> p h f", h=8),
                                    in0=aggp[:, 0:256].rearrange(
                                        "p (h f) -> p h f", h=8),
                                    in1=rec[:].unsqueeze(2)
                                        .broadcast_to([128, 8, 32]),
                                    op=ALU.mult)
                                nc.vector.tensor_add(osb[:], osb[:],
                                                     bias_sb[:])
                                nc.sync.dma_start(
                                    outt[b * 128:(b + 1) * 128, :], osb[:])
    nc.compile()
    return nc


def _prep(x, edge_index, W, W1, W2, att, bias):
    x = np.asarray(x, np.float32)
    ei = np.asarray(edge_index)
    W = np.asarray(W, np.float32)
    W1 = np.asarray(W1, np.float32)
    W2 = np.asarray(W2, np.float32)
    att = np.asarray(att, np.float32)
    bias = np.asarray(bias, np.float32)

    src = ei[0].astype(np.int64)
    dst = ei[1].astype(np.int64)
    perm = np.argsort(dst, kind='stable')
    src_s = src[perm].astype(np.int32)
    dst_s = dst[perm].astype(np.int32)

    core_of = dst_s // NPC
    lblk = (dst_s - core_of * NPC) // 128
    cnt = np.zeros((CORES, BLOCKS), np.int64)
    np.add.at(cnt, (core_of, lblk), 1)
    T_blk = int(np.ceil(cnt.max() / 128))
    if T_blk % 2:
        T_blk += 1
    n_tiles = BLOCKS * T_blk
    n_super = n_tiles // 8

    # padded per-core edge arrays (edges sorted by dst -> contiguous ranges)
    srcc = np.zeros((CORES, n_tiles * 128), np.int32)
    qdst = np.full((CORES, n_tiles * 128), -1.0, np.float32)
    for k in range(CORES):
        for b in range(BLOCKS):
            c = cnt[k, b]
            if c == 0:
                continue
            lo = np.searchsorted(dst_s, k * NPC + b * 128)
            hi = lo + c
            base = b * T_blk * 128
            srcc[k, base:base + c] = src_s[lo:hi]
            qdst[k, base:base + c] = (dst_s[lo:hi] - k * NPC
                                      - b * 128).astype(np.float32)

    # weights, bf16: wcat = [W1.T | W.T], w2T
    wcat = np.concatenate([W1.T, W.T], axis=1).astype(bf16)
    w2T = np.ascontiguousarray(W2.T).astype(bf16)

    # att blocks (unscaled; Prelu handles the leaky slope exactly)
    attb = np.zeros((128, 16), np.float32)
    for p in range(128):
        attb[p, p // 32] = att[0, p // 32, p % 32]
        attb[p, 8 + 4 + p // 32] = att[0, 4 + p // 32, p % 32]
    attb = attb.astype(bf16)

    x_pad = np.zeros((NPADG, IN_F), np.float32)
    x_pad[:N] = x
    xT = np.ascontiguousarray(x_pad.T).astype(bf16)
    iop = np.tile(np.arange(128, dtype=np.float32)[:, None],
                  (1, 1024)).astype(bf16)
    iotaf = np.tile(np.arange(128, dtype=np.float32), (128, 8)).astype(bf16)
    biasr = np.tile(bias[None, :], (128, 1)).astype(np.float32)
    ident = np.eye(128, dtype=np.float32).astype(bf16)

    in_maps = []
    for k in range(CORES):
        xl = np.ascontiguousarray(
            x_pad[k * NPC:k * NPC + NLOC].T).astype(bf16)
        # int16 wrapped indices: super s, local i -> [i%16, s*64 + i//16]
        sw = srcc[k].astype(np.int16).reshape(n_super, 64, 16)
        srcw = np.zeros((128, n_super * 64), np.int16)
        srcw[:16, :] = sw.transpose(2, 0, 1).reshape(16, n_super * 64)
        srcw[16:, :] = np.tile(srcw[:16, :], (7, 1))
        in_maps.append({
            "xt": xT, "xtl": xl, "wcat": wcat, "w2t": w2T,
            "attb": attb, "ident": ident, "iop": iop,
            "iotaf": iotaf, "biasr": biasr,
            "qdsb": np.ascontiguousarray(
                qdst[k].reshape(n_tiles, 128).T).astype(bf16),
            "qdrow": qdst[k][None, :].astype(bf16),
            "srcw": srcw,
        })
    return T_blk, in_maps


def kernel(x, edge_index, W, W1, W2, att, bias):
    global LAST_EXEC_NS
    from concourse import bass_utils

    T_blk, in_maps = _prep(x, edge_index, W, W1, W2, att, bias)
    if T_blk not in _CACHE:
        _CACHE[T_blk] = _build(T_blk)
    nc = _CACHE[T_blk]

    trace = bool(int(os.environ.get("GAT_TRACE", "0")))
    res = bass_utils.run_bass_kernel_spmd(
        nc, in_maps, core_ids=list(range(CORES)), trace=trace)
    LAST_EXEC_NS = res.exec_time_ns

    out = np.empty((N, HF), np.float32)
    for k in range(CORES):
        out[k * NPC:(k + 1) * NPC] = res.results[k]["out"][:NPC]
    return out


# revision 12
# speedup vs baseline: 1.7213x; 1.0101x over previous
"""GATv2 layer on 8 Trainium2 NeuronCores (Bass/Tile) — v3.

Strategy: sort edges by dst on host; core k owns dst nodes [2500k, 2500(k+1))
so softmax + aggregation are core-local. Edges bucketed into 20 blocks of 128
dst nodes per core, padded per block to T_blk tiles of 128 edges (uniform so
one SPMD program serves all cores). 8 tiles = 1 super (1024 edges).

Math notes:
 - logit l_e = att·leaky_relu(hs[src]+hd[dst], 0.2). The ACT engine's Prelu
   (parametric_relu, alpha=0.2) IS leaky relu, so no Relu/linear split.
 - Any per-dst-node additive logit term cancels in the segment softmax
   (exp factor cancels numerator/denominator), so no seg-max subtraction.

Device phases (per core):
  phase 1 (bf16): node table hsv_d[n] = [x@W1.T | x@W.T] (20096 x 512 bf16,
    1KB rows, staged 8 node-tiles per DMA); hd = x@W2.T for the 2560 local
    dst nodes (SBUF-resident).
  phase 2, per super (1024 edges):
    - ONE dma_gather (1024 idx, 1KB rows): hsv_g [128, 8, 512] edge-major.
    - qdT = partition_broadcast(dst-id row) [Pool]; ohT = is_equal(qdT,
      iota_p) [DVE, node-on-partition one-hot]; oh = is_equal(qd, iota_f)
      [Pool, edge-major one-hot].
    - per pair (2 tiles): zT PSUM = PE-transpose(hs slices) + hd[blk]@ohT;
      lr = Prelu(zT) [ACT]; lgt[128,16] = lr@attb (4 small matmuls, full
      PE-array util); exp [ACT]; wv = v*exp [DVE, exp appended for denoms];
      segment-sum via oh-matmul accumulated in PSUM per dst block;
      at block end: normalize by denominator + bias, DMA out.

Cost-model (TimelineSim InstructionCostModel): ~300us/core vs 634us baseline
(Pool-serialized indirect DMAs at 994ns fixed each). Walls: DMA ~217us,
DVE ~195, Pool ~195, ACT ~180, PE ~165.
"""
import os
import sys

sys.path.insert(0, '/opt/trn_rl_repo')

import numpy as np
import ml_dtypes

N = 20000
IN_F = 128
HEADS = 8
OUT_F = 32
HF = 256          # HEADS * OUT_F
NEG = 0.2
CORES = 8
NPC = 2500        # dst nodes per core
BLOCKS = 20       # 128-node blocks per core
NLOC = BLOCKS * 128
NT_GLOB = 157     # ceil(20000/128)
NPADG = NT_GLOB * 128

bf16 = ml_dtypes.bfloat16

_CACHE = {}
LAST_EXEC_NS = None


def _build(T_blk):
    import concourse.bass as bass
    from concourse import mybir, bacc
    from concourse.tile import TileContext

    f32 = mybir.dt.float32
    b16 = mybir.dt.bfloat16
    i16 = mybir.dt.int16
    AF = mybir.ActivationFunctionType
    ALU = mybir.AluOpType

    n_tiles = BLOCKS * T_blk
    n_super = n_tiles // 8

    nc = bacc.Bacc("TRN2", target_bir_lowering=False, debug=False,
                   num_devices=CORES)
    xT = nc.dram_tensor("xt", [128, NPADG], b16, kind="ExternalInput")
    xTl = nc.dram_tensor("xtl", [128, NLOC], b16, kind="ExternalInput")
    wcat = nc.dram_tensor("wcat", [128, 512], b16, kind="ExternalInput")
    w2T = nc.dram_tensor("w2t", [128, 256], b16, kind="ExternalInput")
    attb = nc.dram_tensor("attb", [128, 16], b16, kind="ExternalInput")
    ident = nc.dram_tensor("ident", [128, 128], b16, kind="ExternalInput")
    iop = nc.dram_tensor("iop", [128, 1024], b16, kind="ExternalInput")
    iotaf = nc.dram_tensor("iotaf", [128, 1024], b16, kind="ExternalInput")
    qdsb = nc.dram_tensor("qdsb", [128, n_tiles], b16, kind="ExternalInput")
    qdrow = nc.dram_tensor("qdrow", [1, n_tiles * 128], b16,
                           kind="ExternalInput")
    srcw = nc.dram_tensor("srcw", [128, n_super * 64], i16,
                          kind="ExternalInput")
    biasr = nc.dram_tensor("biasr", [128, 256], f32, kind="ExternalInput")
    outt = nc.dram_tensor("out", [NLOC, 256], f32, kind="ExternalOutput")

    hsv_d = nc.dram_tensor("hsvd", [NPADG, 512], b16, kind="Internal")

    with TileContext(nc) as tc:
        with tc.tile_pool(name="const", bufs=1) as cp:
            wcat_sb = cp.tile([128, 512], b16)
            nc.sync.dma_start(wcat_sb[:], wcat[:])
            w2_sb = cp.tile([128, 256], b16)
            nc.sync.dma_start(w2_sb[:], w2T[:])
            attb_sb = cp.tile([128, 16], b16)
            nc.sync.dma_start(attb_sb[:], attb[:])
            id_sb = cp.tile([128, 128], b16)
            nc.sync.dma_start(id_sb[:], ident[:])
            iop_sb = cp.tile([128, 1024], b16)
            nc.sync.dma_start(iop_sb[:], iop[:])
            iof_sb = cp.tile([128, 1024], b16)
            nc.sync.dma_start(iof_sb[:], iotaf[:])
            qd_sb = cp.tile([128, n_tiles], b16)
            nc.sync.dma_start(qd_sb[:], qdsb[:])
            src_sb = cp.tile([128, n_super * 64], i16)
            nc.sync.dma_start(src_sb[:], srcw[:])
            bias_sb = cp.tile([128, 256], f32)
            nc.sync.dma_start(bias_sb[:], biasr[:])

            # ---------------- phase 1: projections (bf16) ----------------
            NJC = 20    # node tiles per x chunk
            NST = 8     # node tiles per staging/write batch
            with tc.tile_pool(name="proj", bufs=2) as pp, \
                 tc.tile_pool(name="pst", bufs=2) as pstp, \
                 tc.tile_pool(name="pps", bufs=4, space="PSUM") as pps:
                cast_rr = 0
                stage = None
                for j in range(NT_GLOB):
                    if j % NJC == 0:
                        xc = pp.tile([128, 2560], b16, tag="xc")
                        hi = min((j + NJC) * 128, NPADG)
                        nc.sync.dma_start(xc[:, 0:hi - j * 128],
                                          xT[:, j * 128:hi])
                    if j % NST == 0:
                        stage = pstp.tile([128, NST * 512], b16, tag="st")
                    xcol = (j % NJC) * 128
                    ps = pps.tile([128, 512], f32)
                    nc.tensor.matmul(ps[:], lhsT=xc[:, xcol:xcol + 128],
                                     rhs=wcat_sb[:], start=True, stop=True)
                    dstsl = stage[:, (j % NST) * 512:(j % NST) * 512 + 512]
                    if cast_rr == 0:
                        nc.scalar.copy(dstsl, ps[:])
                    else:
                        nc.vector.tensor_copy(dstsl, ps[:])
                    cast_rr = (cast_rr + 1) % 2
                    if j % NST == NST - 1 or j == NT_GLOB - 1:
                        j0 = (j // NST) * NST
                        nt = j - j0 + 1
                        nc.sync.dma_start(
                            hsv_d[j0 * 128:(j0 + nt) * 128, :].rearrange(
                                "(t p) f -> p t f", p=128),
                            stage[:, 0:nt * 512].rearrange(
                                "p (t f) -> p t f", f=512))
                hd_tiles = []
                xl = pp.tile([128, NLOC], b16, tag="xl")
                nc.sync.dma_start(xl[:], xTl[:])
                for b in range(BLOCKS):
                    ps2 = pps.tile([128, 256], f32)
                    nc.tensor.matmul(ps2[:], lhsT=xl[:, b * 128:b * 128 + 128],
                                     rhs=w2_sb[:], start=True, stop=True)
                    hdt = cp.tile([128, 256], b16, tag=f"hd{b}")
                    if b % 2 == 0:
                        nc.scalar.copy(hdt[:], ps2[:])
                    else:
                        nc.vector.tensor_copy(hdt[:], ps2[:])
                    hd_tiles.append(hdt)

            # ---------------- phase 2: edges ----------------
            with tc.tile_pool(name="edge", bufs=2) as ep, \
                 tc.tile_pool(name="pair", bufs=3) as rp, \
                 tc.tile_pool(name="zps", bufs=2, space="PSUM") as zps, \
                 tc.tile_pool(name="lps", bufs=2, space="PSUM") as lps, \
                 tc.tile_pool(name="aps", bufs=2, space="PSUM") as aps, \
                 tc.tile_pool(name="apd", bufs=2, space="PSUM") as apd, \
                 tc.tile_pool(name="np_", bufs=2) as npl:
                aggp = None
                aggd = None
                for s in range(n_super):
                    idsl = src_sb[:, s * 64:(s + 1) * 64]
                    hsv_g = ep.tile([128, 8 * 512], b16, tag="hsvg")
                    nc.gpsimd.dma_gather(
                        hsv_g[:].rearrange("p (t f) -> p t f", f=512),
                        hsv_d[:], idsl, 1024, 1024, 512,
                        transpose=False)
                    qstage = ep.tile([128, 1024], b16, tag="qst")
                    nc.sync.dma_start(qstage[0:1, :],
                                      qdrow[:, s * 1024:(s + 1) * 1024])
                    qdT = ep.tile([128, 1024], b16, tag="qdT")
                    nc.gpsimd.partition_broadcast(qdT[:], qstage[0:1, :])
                    ohT = ep.tile([128, 1024], b16, tag="ohT")
                    nc.vector.tensor_tensor(out=ohT[:], in0=qdT[:],
                                            in1=iop_sb[:], op=ALU.is_equal)
                    oh = ep.tile([128, 1024], b16, tag="oh")
                    nc.vector.tensor_tensor(
                        out=oh[:].rearrange("p (t c) -> p t c", t=8),
                        in0=qd_sb[:, s * 8:(s + 1) * 8].unsqueeze(2)
                            .broadcast_to([128, 8, 128]),
                        in1=iof_sb[:].rearrange("p (t c) -> p t c", t=8),
                        op=ALU.is_equal)
                    for q in range(4):   # pairs within super
                        t0 = s * 8 + 2 * q
                        blk = t0 // T_blk
                        hdt = hd_tiles[blk]
                        # zT [128, (hf, sl, e)] in PSUM; per 128-col region:
                        # transpose(hs) opens the accum group, hd one-hot
                        # closes it (groups strictly sequential per region)
                        psZ = zps.tile([128, 512], f32)
                        for hf in range(2):
                            for sl in range(2):
                                reg = psZ[:, hf * 256 + sl * 128:
                                          hf * 256 + sl * 128 + 128]
                                nc.tensor.matmul(
                                    reg,
                                    lhsT=hsv_g[:, (2 * q + sl) * 512 +
                                               hf * 128:
                                               (2 * q + sl) * 512 +
                                               hf * 128 + 128],
                                    rhs=id_sb[:], start=True, stop=False)
                                nc.tensor.matmul(
                                    reg,
                                    lhsT=hdt[:, hf * 128:hf * 128 + 128],
                                    rhs=ohT[:, (2 * q + sl) * 128:
                                            (2 * q + sl) * 128 + 128],
                                    start=False, stop=True)
                        lr = rp.tile([128, 512], b16, tag="lr")
                        nc.scalar.activation(lr[:], psZ[:], AF.Prelu,
                                             alpha=NEG)
                        lgt = lps.tile([128, 16], f32)
                        for sl in range(2):
                            for hf in range(2):
                                nc.tensor.matmul(
                                    lgt[:, sl * 8:sl * 8 + 8],
                                    lhsT=lr[:, hf * 256 + sl * 128:
                                            hf * 256 + sl * 128 + 128],
                                    rhs=attb_sb[:, hf * 8:hf * 8 + 8],
                                    start=(hf == 0), stop=(hf == 1))
                        wvp = rp.tile([128, 528], b16, tag="wvp")
                        nc.scalar.activation(
                            wvp[:].rearrange("p (t c) -> p t c", t=2)
                                [:, :, 256:264],
                            lgt[:].rearrange("p (a b) -> p a b", a=2),
                            AF.Exp)
                        nc.vector.tensor_tensor(
                            out=wvp[:].rearrange("p (t c) -> p t c", t=2)
                                [:, :, 0:256].rearrange(
                                    "p t (f h) -> p t f h", h=8),
                            in0=hsv_g[:].rearrange("p (t f) -> p t f", f=512)
                                [:, 2 * q:2 * q + 2, 256:512].rearrange(
                                    "p t (f h) -> p t f h", h=8),
                            in1=wvp[:].rearrange("p (t c) -> p t c", t=2)
                                [:, :, 256:264].unsqueeze(2)
                                .broadcast_to([128, 2, 32, 8]),
                            op=ALU.mult)
                        for sl in range(2):
                            t = t0 + sl
                            tb = t % T_blk
                            ohsl = oh[:, (2 * q + sl) * 128:
                                      (2 * q + sl) * 128 + 128]
                            if tb == 0:
                                aggp = aps.tile([128, 256], f32, tag="agg")
                                aggd = apd.tile([128, 8], f32, tag="aggd")
                            nc.tensor.matmul(
                                aggp[:],
                                lhsT=ohsl,
                                rhs=wvp[:, sl * 264:sl * 264 + 256]
                                    .rearrange("p (f h) -> p h f", h=8),
                                start=(tb == 0), stop=(tb == T_blk - 1))
                            nc.tensor.matmul(
                                aggd[:],
                                lhsT=ohsl,
                                rhs=wvp[:, sl * 264 + 256:sl * 264 + 264],
                                start=(tb == 0), stop=(tb == T_blk - 1))
                            if tb == T_blk - 1:
                                b = t // T_blk
                                dn = npl.tile([128, 8], f32, tag="dn")
                                nc.vector.tensor_scalar(
                                    out=dn[:], in0=aggd[:],
                                    scalar1=1e-12, scalar2=None, op0=ALU.max)
                                rec = npl.tile([128, 8], f32, tag="rec")
                                nc.vector.reciprocal(rec[:], dn[:])
                                osb = npl.tile([128, 256], f32, tag="osb")
                                nc.vector.tensor_tensor(
                                    out=osb[:].rearrange(
                                        "p (h f) -> p h f", h=8),
                                    in0=aggp[:].rearrange(
                                        "p (h f) -> p h f", h=8),
                                    in1=rec[:].unsqueeze(2)
                                        .broadcast_to([128, 8, 32]),
                                    op=ALU.mult)
                                nc.vector.tensor_add(osb[:], osb[:],
                                                     bias_sb[:])
                                nc.sync.dma_start(
                                    outt[b * 128:(b + 1) * 128, :], osb[:])
    nc.compile()
    return nc


def _prep(x, edge_index, W, W1, W2, att, bias):
    x = np.asarray(x, np.float32)
    ei = np.asarray(edge_index)
    W = np.asarray(W, np.float32)
    W1 = np.asarray(W1, np.float32)
    W2 = np.asarray(W2, np.float32)
    att = np.asarray(att, np.float32)
    bias = np.asarray(bias, np.float32)

    src = ei[0].astype(np.int64)
    dst = ei[1].astype(np.int64)
    perm = np.argsort(dst, kind='stable')
    src_s = src[perm].astype(np.int32)
    dst_s = dst[perm].astype(np.int32)

    core_of = dst_s // NPC
    lblk = (dst_s - core_of * NPC) // 128
    cnt = np.zeros((CORES, BLOCKS), np.int64)
    np.add.at(cnt, (core_of, lblk), 1)
    T_blk = int(np.ceil(cnt.max() / 128))
    if T_blk % 2:
        T_blk += 1
    n_tiles = BLOCKS * T_blk
    n_super = n_tiles // 8

    # padded per-core edge arrays (edges sorted by dst -> contiguous ranges)
    srcc = np.zeros((CORES, n_tiles * 128), np.int32)
    qdst = np.full((CORES, n_tiles * 128), -1.0, np.float32)
    for k in range(CORES):
        for b in range(BLOCKS):
            c = cnt[k, b]
            if c == 0:
                continue
            lo = np.searchsorted(dst_s, k * NPC + b * 128)
            hi = lo + c
            base = b * T_blk * 128
            srcc[k, base:base + c] = src_s[lo:hi]
            qdst[k, base:base + c] = (dst_s[lo:hi] - k * NPC
                                      - b * 128).astype(np.float32)

    # weights, bf16: wcat = [W1.T | W.T-(f,h)-interleaved], w2T
    # v columns stored in (feature, head) order so the wv multiply's exp
    # broadcast is packed on its last axis (DVE 2x mode); the aggregation
    # matmul un-permutes via a strided rhs read.
    vperm = (np.arange(8)[None, :] * 32
             + np.arange(32)[:, None]).reshape(256)
    wcat = np.concatenate([W1.T, W.T[:, vperm]], axis=1).astype(bf16)
    w2T = np.ascontiguousarray(W2.T).astype(bf16)

    # att blocks (unscaled; Prelu handles the leaky slope exactly)
    attb = np.zeros((128, 16), np.float32)
    for p in range(128):
        attb[p, p // 32] = att[0, p // 32, p % 32]
        attb[p, 8 + 4 + p // 32] = att[0, 4 + p // 32, p % 32]
    attb = attb.astype(bf16)

    x_pad = np.zeros((NPADG, IN_F), np.float32)
    x_pad[:N] = x
    xT = np.ascontiguousarray(x_pad.T).astype(bf16)
    iop = np.tile(np.arange(128, dtype=np.float32)[:, None],
                  (1, 1024)).astype(bf16)
    iotaf = np.tile(np.arange(128, dtype=np.float32), (128, 8)).astype(bf16)
    biasr = np.tile(bias[None, :], (128, 1)).astype(np.float32)
    ident = np.eye(128, dtype=np.float32).astype(bf16)

    in_maps = []
    for k in range(CORES):
        xl = np.ascontiguousarray(
            x_pad[k * NPC:k * NPC + NLOC].T).astype(bf16)
        # int16 wrapped indices: super s, local i -> [i%16, s*64 + i//16]
        sw = srcc[k].astype(np.int16).reshape(n_super, 64, 16)
        srcw = np.zeros((128, n_super * 64), np.int16)
        srcw[:16, :] = sw.transpose(2, 0, 1).reshape(16, n_super * 64)
        srcw[16:, :] = np.tile(srcw[:16, :], (7, 1))
        in_maps.append({
            "xt": xT, "xtl": xl, "wcat": wcat, "w2t": w2T,
            "attb": attb, "ident": ident, "iop": iop,
            "iotaf": iotaf, "biasr": biasr,
            "qdsb": np.ascontiguousarray(
                qdst[k].reshape(n_tiles, 128).T).astype(bf16),
            "qdrow": qdst[k][None, :].astype(bf16),
            "srcw": srcw,
        })
    return T_blk, in_maps


def kernel(x, edge_index, W, W1, W2, att, bias):
    global LAST_EXEC_NS
    from concourse import bass_utils

    T_blk, in_maps = _prep(x, edge_index, W, W1, W2, att, bias)
    if T_blk not in _CACHE:
        _CACHE[T_blk] = _build(T_blk)
    nc = _CACHE[T_blk]

    trace = bool(int(os.environ.get("GAT_TRACE", "0")))
    res = bass_utils.run_bass_kernel_spmd(
        nc, in_maps, core_ids=list(range(CORES)), trace=trace)
    LAST_EXEC_NS = res.exec_time_ns

    out = np.empty((N, HF), np.float32)
    for k in range(CORES):
        out[k * NPC:(k + 1) * NPC] = res.results[k]["out"][:NPC]
    return out


# revision 30
# speedup vs baseline: 2.1266x; 1.2355x over previous
"""GATv2 layer on 8 Trainium2 NeuronCores (Bass/Tile) — v4.

Strategy: sort edges by dst on host; core k owns dst nodes [2500k, 2500(k+1))
so softmax + aggregation are core-local. Edges grouped per 128-dst block.

Phase overlap: the node table is split in two halves (hsv_a: node tiles
0..78, hsv_b: 79..156, separate DRAM tensors so the tile framework sees
clean dependencies). Each block's edges are partitioned into A-edges
(src in first half) and B-edges; A-supers (8 tiles of 128 edges) only
gather from hsv_a, so their edge processing overlaps the projection of
the B half (emission explicitly interleaves B-projection tiles between
A-supers). Aggregation is two-stage: each block's A-partial (numerator +
softmax denominator) is checkpointed to SBUF (f32) when its A-tiles end,
and added back into the B aggregation PSUM via an identity matmul.

Math notes:
 - logit l_e = att·leaky_relu(hs[src]+hd[dst], 0.2). The ACT engine's Prelu
   (parametric_relu, alpha=0.2) IS leaky relu, so no Relu/linear split.
 - Any per-dst-node additive logit term cancels in the segment softmax,
   so no seg-max subtraction is needed (logits are O(+-5) here).
 - v columns are stored (feature, head)-interleaved so the wv multiply's
   exp broadcast is packed on the last axis (DVE 2x mode); the aggregation
   matmul un-permutes via a strided rhs read.

Per super (1024 edges): one dma_gather (1024 idx, 1KB rows) -> hsv_g
edge-major; qdT = partition_broadcast(dst row) on Pool; ohT/oh one-hots by
is_equal on DVE; per pair of tiles: zT PSUM = PE-transpose(hs) + hd@ohT
(accum groups strictly sequential per 128-col region); lr = Prelu(zT);
logits = lr@attb (edge-major, full PE-array util); exp; wv = v*exp;
segment-sum via oh-matmul accumulated in PSUM per (block, half); block end:
normalize by denominator + bias, DMA out.
"""
import os
import sys

sys.path.insert(0, '/opt/trn_rl_repo')

import numpy as np
import ml_dtypes

N = 20000
IN_F = 128
HEADS = 8
OUT_F = 32
HF = 256          # HEADS * OUT_F
NEG = 0.2
CORES = 8
NPC = 2500        # dst nodes per core
BLOCKS = 20       # 128-node blocks per core
NLOC = BLOCKS * 128
NT_GLOB = 157     # ceil(20000/128)
NPADG = NT_GLOB * 128
NHA_T = 79        # node tiles in half A
NHA = NHA_T * 128             # 10112
NHB = (NT_GLOB - NHA_T) * 128  # 9984

bf16 = ml_dtypes.bfloat16

_CACHE = {}
LAST_EXEC_NS = None


def _build(structure):
    import concourse.bass as bass
    from concourse import mybir, bacc
    from concourse.tile import TileContext

    f32 = mybir.dt.float32
    b16 = mybir.dt.bfloat16
    i16 = mybir.dt.int16
    AF = mybir.ActivationFunctionType
    ALU = mybir.AluOpType

    SA, supers = structure
    n_super = len(supers)
    n_tiles = 8 * n_super

    nc = bacc.Bacc("TRN2", target_bir_lowering=False, debug=False,
                   num_devices=CORES)
    xT = nc.dram_tensor("xt", [128, NPADG], b16, kind="ExternalInput")
    xTl = nc.dram_tensor("xtl", [128, NLOC], b16, kind="ExternalInput")
    wcat = nc.dram_tensor("wcat", [128, 512], b16, kind="ExternalInput")
    w2T = nc.dram_tensor("w2t", [128, 256], b16, kind="ExternalInput")
    attb = nc.dram_tensor("attb", [128, 16], b16, kind="ExternalInput")
    ident = nc.dram_tensor("ident", [128, 128], b16, kind="ExternalInput")
    identf = nc.dram_tensor("identf", [128, 128], f32, kind="ExternalInput")
    iop = nc.dram_tensor("iop", [128, 1024], b16, kind="ExternalInput")
    iotaf = nc.dram_tensor("iotaf", [128, 1024], b16, kind="ExternalInput")
    qdsb = nc.dram_tensor("qdsb", [128, n_tiles], b16, kind="ExternalInput")
    qdrow = nc.dram_tensor("qdrow", [1, n_tiles * 128], b16,
                           kind="ExternalInput")
    srcw = nc.dram_tensor("srcw", [128, n_super * 64], i16,
                          kind="ExternalInput")
    biasr = nc.dram_tensor("biasr", [128, 256], f32, kind="ExternalInput")
    outt = nc.dram_tensor("out", [NLOC, 256], f32, kind="ExternalOutput")

    hsv_a = nc.dram_tensor("hsva", [NHA, 512], b16, kind="Internal")
    hsv_b = nc.dram_tensor("hsvb", [NHB, 512], b16, kind="Internal")

    with TileContext(nc) as tc:
        with tc.tile_pool(name="const", bufs=1) as cp:
            wcat_sb = cp.tile([128, 512], b16)
            nc.sync.dma_start(wcat_sb[:], wcat[:])
            w2_sb = cp.tile([128, 256], b16)
            nc.sync.dma_start(w2_sb[:], w2T[:])
            attb_sb = cp.tile([128, 16], b16)
            nc.sync.dma_start(attb_sb[:], attb[:])
            id_sb = cp.tile([128, 128], b16)
            nc.sync.dma_start(id_sb[:], ident[:])
            idf_sb = cp.tile([128, 128], f32)
            nc.sync.dma_start(idf_sb[:], identf[:])
            iop_sb = cp.tile([128, 1024], b16)
            nc.sync.dma_start(iop_sb[:], iop[:])
            iof_sb = cp.tile([128, 1024], b16)
            nc.sync.dma_start(iof_sb[:], iotaf[:])
            qd_sb = cp.tile([128, n_tiles], b16)
            nc.sync.dma_start(qd_sb[:], qdsb[:])
            src_sb = cp.tile([128, n_super * 64], i16)
            nc.sync.dma_start(src_sb[:], srcw[:])
            bias_sb = cp.tile([128, 256], f32)
            nc.sync.dma_start(bias_sb[:], biasr[:])
            stA = []
            for b in range(BLOCKS):
                _t = cp.tile([128, 264], f32, tag=f"stA{b}",
                             name=f"stA{b}")
                stA.append(_t)
            hd_tiles = [None] * BLOCKS

            NJC = 20    # node tiles per x chunk
            NST = 8     # node tiles per staging/write batch

            # ---------------- phase 1a: half-A projections + hd ----------
            with tc.tile_pool(name="proj", bufs=2) as pp, \
                 tc.tile_pool(name="pst", bufs=3) as pstp, \
                 tc.tile_pool(name="pps", bufs=6, space="PSUM") as pps, \
                 tc.tile_pool(name="pps2", bufs=2, space="PSUM") as pps2:
                xl = pp.tile([128, NLOC], b16, tag="xl")
                nc.sync.dma_start(xl[:], xTl[:])
                cast_rr = 0
                stage = None
                for j in range(NHA_T):
                    if j % NJC == 0:
                        xc = pp.tile([128, 2560], b16, tag="xc")
                        hi = min((j + NJC) * 128, NHA)
                        nc.sync.dma_start(xc[:, 0:hi - j * 128],
                                          xT[:, j * 128:hi])
                    if j % NST == 0:
                        stage = pstp.tile([128, NST * 512], b16, tag="st")
                    xcol = (j % NJC) * 128
                    ps = pps.tile([128, 512], f32)
                    nc.tensor.matmul(ps[:], lhsT=xc[:, xcol:xcol + 128],
                                     rhs=wcat_sb[:], start=True, stop=True)
                    dstsl = stage[:, (j % NST) * 512:(j % NST) * 512 + 512]
                    if cast_rr == 0:
                        nc.scalar.copy(dstsl, ps[:])
                    else:
                        nc.vector.tensor_copy(dstsl, ps[:])
                    cast_rr = (cast_rr + 1) % 2
                    if j % NST == NST - 1 or j == NHA_T - 1:
                        j0 = (j // NST) * NST
                        nt = j - j0 + 1
                        nc.sync.dma_start(
                            hsv_a[j0 * 128:(j0 + nt) * 128, :].rearrange(
                                "(t p) f -> p t f", p=128),
                            stage[:, 0:nt * 512].rearrange(
                                "p (t f) -> p t f", f=512))
                    # one local-dst (hd) tile every 4 node tiles
                    if j % 4 == 3 and j // 4 < BLOCKS:
                        b = j // 4
                        ps2 = pps2.tile([128, 256], f32)
                        nc.tensor.matmul(ps2[:],
                                         lhsT=xl[:, b * 128:b * 128 + 128],
                                         rhs=w2_sb[:], start=True, stop=True)
                        hdt = cp.tile([128, 256], b16, tag=f"hd{b}")
                        if b % 2 == 0:
                            nc.scalar.copy(hdt[:], ps2[:])
                        else:
                            nc.vector.tensor_copy(hdt[:], ps2[:])
                        hd_tiles[b] = hdt
                for b in range(BLOCKS):
                    if hd_tiles[b] is not None:
                        continue
                    ps2 = pps2.tile([128, 256], f32)
                    nc.tensor.matmul(ps2[:],
                                     lhsT=xl[:, b * 128:b * 128 + 128],
                                     rhs=w2_sb[:], start=True, stop=True)
                    hdt = cp.tile([128, 256], b16, tag=f"hd{b}")
                    nc.vector.tensor_copy(hdt[:], ps2[:])
                    hd_tiles[b] = hdt

            # ------- phase 1b (half-B projections) + phase 2, interleaved --
            with tc.tile_pool(name="projB", bufs=2) as ppB, \
                 tc.tile_pool(name="pstB", bufs=2) as pstB, \
                 tc.tile_pool(name="ppsB", bufs=2, space="PSUM") as ppsB, \
                 tc.tile_pool(name="edge", bufs=5) as ep, \
                 tc.tile_pool(name="edgeq", bufs=3) as epq, \
                 tc.tile_pool(name="pair", bufs=6) as rp, \
                 tc.tile_pool(name="zps", bufs=2, space="PSUM") as zps, \
                 tc.tile_pool(name="lps", bufs=2, space="PSUM") as lps, \
                 tc.tile_pool(name="aps", bufs=1, space="PSUM") as aps, \
                 tc.tile_pool(name="apd", bufs=1, space="PSUM") as apd, \
                 tc.tile_pool(name="np_", bufs=2) as npl:

                bstate = {"xc": None, "stage": None, "cast_rr": 0}

                def emit_b_tile(jb):
                    # node tile NHA_T + jb -> hsv_b rows jb*128..
                    j = NHA_T + jb
                    if jb % NJC == 0:
                        bstate["xc"] = ppB.tile([128, 2560], b16, tag="xcB", name="xcB")
                        hi = min((jb + NJC) * 128 + NHA, NPADG)
                        nc.sync.dma_start(
                            bstate["xc"][:, 0:hi - j * 128],
                            xT[:, j * 128:hi])
                    if jb % NST == 0:
                        bstate["stage"] = pstB.tile([128, NST * 512], b16,
                                                    tag="stB", name="stB")
                    xcol = (jb % NJC) * 128
                    ps = ppsB.tile([128, 512], f32)
                    nc.tensor.matmul(ps[:],
                                     lhsT=bstate["xc"][:, xcol:xcol + 128],
                                     rhs=wcat_sb[:], start=True, stop=True)
                    dstsl = bstate["stage"][:, (jb % NST) * 512:
                                            (jb % NST) * 512 + 512]
                    if bstate["cast_rr"] == 0:
                        nc.scalar.copy(dstsl, ps[:])
                    else:
                        nc.vector.tensor_copy(dstsl, ps[:])
                    bstate["cast_rr"] = (bstate["cast_rr"] + 1) % 2
                    nb_t = NT_GLOB - NHA_T
                    if jb % NST == NST - 1 or jb == nb_t - 1:
                        j0 = (jb // NST) * NST
                        nt = jb - j0 + 1
                        nc.sync.dma_start(
                            hsv_b[j0 * 128:(j0 + nt) * 128, :].rearrange(
                                "(t p) f -> p t f", p=128),
                            bstate["stage"][:, 0:nt * 512].rearrange(
                                "p (t f) -> p t f", f=512))

                agst = {"aggp": None, "aggd": None}

                def normalize_out(b):
                    dn = npl.tile([128, 8], f32, tag="dn")
                    nc.vector.tensor_scalar(
                        out=dn[:], in0=agst["aggd"][:],
                        scalar1=1e-12, scalar2=None, op0=ALU.max)
                    rec = npl.tile([128, 8], f32, tag="rec")
                    nc.vector.reciprocal(rec[:], dn[:])
                    osb = npl.tile([128, 256], f32, tag="osb")
                    nc.vector.tensor_tensor(
                        out=osb[:].rearrange("p (h f) -> p h f", h=8),
                        in0=agst["aggp"][:].rearrange("p (h f) -> p h f", h=8),
                        in1=rec[:].unsqueeze(2).broadcast_to([128, 8, 32]),
                        op=ALU.mult)
                    nc.vector.tensor_add(osb[:], osb[:], bias_sb[:])
                    nc.sync.dma_start(outt[b * 128:(b + 1) * 128, :], osb[:])

                def process_super(s, sup):
                    table = hsv_a if s < SA else hsv_b
                    idsl = src_sb[:, s * 64:(s + 1) * 64]
                    hsv_g = ep.tile([128, 8 * 512], b16, tag="hsvg")
                    nc.gpsimd.dma_gather(
                        hsv_g[:].rearrange("p (t f) -> p t f", f=512),
                        table[:], idsl, 1024, 1024, 512,
                        transpose=False)
                    qstage = epq.tile([128, 1024], b16, tag="qst")
                    nc.sync.dma_start(qstage[0:1, :],
                                      qdrow[:, s * 1024:(s + 1) * 1024])
                    qdT = epq.tile([128, 1024], b16, tag="qdT")
                    nc.gpsimd.partition_broadcast(qdT[:], qstage[0:1, :])
                    ohT = ep.tile([128, 1024], b16, tag="ohT")
                    nc.vector.tensor_tensor(out=ohT[:], in0=qdT[:],
                                            in1=iop_sb[:], op=ALU.is_equal)
                    oh = ep.tile([128, 1024], b16, tag="oh")
                    nc.vector.tensor_tensor(
                        out=oh[:].rearrange("p (t c) -> p t c", t=8),
                        in0=qd_sb[:, s * 8:(s + 1) * 8].unsqueeze(2)
                            .broadcast_to([128, 8, 128]),
                        in1=iof_sb[:].rearrange("p (t c) -> p t c", t=8),
                        op=ALU.is_equal)
                    tof = 0
                    for unit in sup:
                        blk, dead, ntl, tflags = unit
                        hdt = hd_tiles[blk]
                        u0 = tof
                        tof += ntl
                        W = 256 * ntl
                        # zT in PSUM; per 128-col region transpose(hs) opens
                        # the accum group, hd one-hot closes it
                        psZ = zps.tile([128, 512], f32, tag="z")
                        for hf in range(2):
                            for sl in range(ntl):
                                reg = psZ[:, hf * 128 * ntl + sl * 128:
                                          hf * 128 * ntl + sl * 128 + 128]
                                nc.tensor.matmul(
                                    reg,
                                    lhsT=hsv_g[:, (u0 + sl) * 512 +
                                               hf * 128:
                                               (u0 + sl) * 512 +
                                               hf * 128 + 128],
                                    rhs=id_sb[:], start=True, stop=False)
                                nc.tensor.matmul(
                                    reg,
                                    lhsT=hdt[:, hf * 128:hf * 128 + 128],
                                    rhs=ohT[:, (u0 + sl) * 128:
                                            (u0 + sl) * 128 + 128],
                                    start=False, stop=True)
                        lr = rp.tile([128, 512], b16, tag="lr")
                        nc.scalar.activation(lr[:, 0:W], psZ[:, 0:W],
                                             AF.Prelu, alpha=NEG)
                        lgt = lps.tile([128, 16], f32)
                        for sl in range(ntl):
                            for hf in range(2):
                                nc.tensor.matmul(
                                    lgt[:, sl * 8:sl * 8 + 8],
                                    lhsT=lr[:, hf * 128 * ntl + sl * 128:
                                            hf * 128 * ntl + sl * 128 + 128],
                                    rhs=attb_sb[:, hf * 8:hf * 8 + 8],
                                    start=(hf == 0), stop=(hf == 1))
                        wvp = rp.tile([128, 528], b16, tag="wvp")
                        nc.scalar.activation(
                            wvp[:].rearrange("p (t c) -> p t c", t=2)
                                [:, 0:ntl, 256:264],
                            lgt[:, 0:8 * ntl].rearrange(
                                "p (a b) -> p a b", b=8),
                            AF.Exp)
                        nc.vector.tensor_tensor(
                            out=wvp[:].rearrange("p (t c) -> p t c", t=2)
                                [:, 0:ntl, 0:256].rearrange(
                                    "p t (f h) -> p t f h", h=8),
                            in0=hsv_g[:].rearrange("p (t f) -> p t f", f=512)
                                [:, u0:u0 + ntl, 256:512].rearrange(
                                    "p t (f h) -> p t f h", h=8),
                            in1=wvp[:].rearrange("p (t c) -> p t c", t=2)
                                [:, 0:ntl, 256:264].unsqueeze(2)
                                .broadcast_to([128, ntl, 32, 8]),
                            op=ALU.mult)
                        if dead:
                            continue
                        for sl in range(ntl):
                            tstart, tstop = tflags[sl]
                            ohsl = oh[:, (u0 + sl) * 128:
                                      (u0 + sl) * 128 + 128]
                            if tstart:
                                agst["aggp"] = aps.tile([128, 256], f32,
                                                        tag="agg", name="agg")
                                agst["aggd"] = apd.tile([128, 8], f32,
                                                        tag="aggd", name="aggd")
                            hardstop = tstop in (1, 3)
                            nc.tensor.matmul(
                                agst["aggp"][:],
                                lhsT=ohsl,
                                rhs=wvp[:, sl * 264:sl * 264 + 256]
                                    .rearrange("p (f h) -> p h f", h=8),
                                start=tstart, stop=hardstop)
                            nc.tensor.matmul(
                                agst["aggd"][:],
                                lhsT=ohsl,
                                rhs=wvp[:, sl * 264 + 256:sl * 264 + 264],
                                start=tstart, stop=hardstop)
                            if tstop == 1:
                                # close A-partial: checkpoint to SBUF (f32)
                                nc.scalar.copy(stA[blk][:, 0:256],
                                               agst["aggp"][:])
                                nc.scalar.copy(stA[blk][:, 256:264],
                                               agst["aggd"][:])
                            elif tstop == 2:
                                # combine A-partial back, then finalize
                                nc.tensor.matmul(
                                    agst["aggp"][:], lhsT=idf_sb[:],
                                    rhs=stA[blk][:, 0:256],
                                    start=False, stop=True)
                                nc.tensor.matmul(
                                    agst["aggd"][:], lhsT=idf_sb[:],
                                    rhs=stA[blk][:, 256:264],
                                    start=False, stop=True)
                                normalize_out(blk)
                            elif tstop == 3:
                                normalize_out(blk)

                nb_t = NT_GLOB - NHA_T
                done_b = 0
                for s in range(SA):
                    share = ((s + 1) * nb_t + SA - 1) // SA
                    while done_b < min(share, nb_t):
                        emit_b_tile(done_b)
                        done_b += 1
                    process_super(s, supers[s])
                while done_b < nb_t:
                    emit_b_tile(done_b)
                    done_b += 1
                for s in range(SA, n_super):
                    process_super(s, supers[s])
    nc.compile()
    return nc


def _prep(x, edge_index, W, W1, W2, att, bias):
    x = np.asarray(x, np.float32)
    ei = np.asarray(edge_index)
    W = np.asarray(W, np.float32)
    W1 = np.asarray(W1, np.float32)
    W2 = np.asarray(W2, np.float32)
    att = np.asarray(att, np.float32)
    bias = np.asarray(bias, np.float32)

    src = ei[0].astype(np.int64)
    dst = ei[1].astype(np.int64)
    perm = np.argsort(dst, kind='stable')
    src_s = src[perm].astype(np.int32)
    dst_s = dst[perm].astype(np.int32)

    core_of = dst_s // NPC
    lblk = (dst_s - core_of * NPC) // 128
    cnt = np.zeros((CORES, BLOCKS), np.int64)
    np.add.at(cnt, (core_of, lblk), 1)

    # per-core tile streams; identical structure required across cores for
    # one SPMD program -> build per-core structures and take the max layout
    per_core = []
    for k in range(CORES):
        tiles = {"A": [], "B": []}   # (blk, src128, qd128, first, last)
        for b in range(BLOCKS):
            c = cnt[k, b]
            lo = np.searchsorted(dst_s, k * NPC + b * 128)
            e_src = src_s[lo:lo + c]
            e_qd = (dst_s[lo:lo + c] - k * NPC - b * 128).astype(np.float32)
            isA = e_src < NHA
            for side in ("A", "B"):
                m = isA if side == "A" else ~isA
                s_arr = e_src[m]
                q_arr = e_qd[m]
                if side == "B":
                    s_arr = s_arr - NHA
                ne = len(s_arr)
                if ne == 0:
                    continue
                T = (ne + 127) // 128
                sp = np.zeros(T * 128, np.int32)
                qp = np.full(T * 128, -1.0, np.float32)
                sp[:ne] = s_arr
                qp[:ne] = q_arr
                for i in range(T):
                    tiles[side].append(
                        (b, sp[i * 128:(i + 1) * 128],
                         qp[i * 128:(i + 1) * 128], i == 0, i == T - 1))
        per_core.append(tiles)

    # SPMD: all cores share one program; pad every core to the max tile
    # count per side and union the stop/start structure. Simplest correct
    # approach: use core-0's structure only if all cores match; otherwise
    # pad each (blk, side) run to the max T across cores.
    # Rebuild with uniform per-(blk, side) tile counts:
    Tmax = {}
    for side in ("A", "B"):
        for b in range(BLOCKS):
            m = 0
            for k in range(CORES):
                tl = [t for t in per_core[k][side] if t[0] == b]
                m = max(m, len(tl))
            Tmax[(side, b)] = m

    supers_struct = []
    tile_arrays = []   # per core appended later
    order = []         # (side, blk, idx_in_run, first, last)
    for side in ("A", "B"):
        for b in range(BLOCKS):
            T = Tmax[(side, b)]
            for i in range(T):
                order.append((side, b, i, i == 0, i == T - 1))
        # pad to super boundary with dead tiles
        while len(order) % 8:
            order.append((side, 0, -1, False, False))
        if side == "A":
            nA_tiles = len(order)
    SA = nA_tiles // 8
    n_tiles = len(order)
    n_super = n_tiles // 8

    # stop codes: per (side, blk): last tile -> 1 (partial) if the other
    # side has tiles for this blk, else 3; side B last -> 2 if A had tiles
    def tile_flags(to):
        side, b, i, first, last = to
        stop = 0
        if last:
            if side == "A":
                stop = 1 if Tmax[("B", b)] > 0 else 3
            else:
                stop = 2 if Tmax[("A", b)] > 0 else 3
        return (bool(first), int(stop))

    for s in range(n_super):
        sup = []
        t = 0
        while t < 8:
            t0o = order[s * 8 + t]
            side0, b0, i0 = t0o[0], t0o[1], t0o[2]
            pair = False
            if t < 7:
                t1o = order[s * 8 + t + 1]
                if (i0 >= 0 and t1o[0] == side0 and t1o[1] == b0
                        and t1o[2] == i0 + 1 and i0 % 2 == 0):
                    pair = True
                elif i0 < 0 and t1o[2] < 0:
                    pair = True
            dead = i0 < 0
            if pair:
                sup.append((int(b0), bool(dead), 2,
                            (tile_flags(t0o), tile_flags(t1o))))
                t += 2
            else:
                sup.append((int(b0), bool(dead), 1,
                            (tile_flags(t0o),)))
                t += 1
        supers_struct.append(tuple(sup))
    structure = (SA, tuple(supers_struct))

    # per-core edge arrays following `order`
    srcc = np.zeros((CORES, n_tiles * 128), np.int32)
    qdst = np.full((CORES, n_tiles * 128), -1.0, np.float32)
    for k in range(CORES):
        runs = {}
        for side in ("A", "B"):
            for t in per_core[k][side]:
                runs.setdefault((side, t[0]), []).append(t)
        for ti, (side, b, i, first, last) in enumerate(order):
            if i < 0:
                continue
            run = runs.get((side, b), [])
            if i < len(run):
                srcc[k, ti * 128:(ti + 1) * 128] = run[i][1]
                qdst[k, ti * 128:(ti + 1) * 128] = run[i][2]

    # weights, bf16: wcat = [W1.T | W.T-(f,h)-interleaved], w2T
    vperm = (np.arange(8)[None, :] * 32
             + np.arange(32)[:, None]).reshape(256)
    wcat = np.concatenate([W1.T, W.T[:, vperm]], axis=1).astype(bf16)
    w2T = np.ascontiguousarray(W2.T).astype(bf16)

    # att blocks (unscaled; Prelu handles the leaky slope exactly)
    attb = np.zeros((128, 16), np.float32)
    for p in range(128):
        attb[p, p // 32] = att[0, p // 32, p % 32]
        attb[p, 8 + 4 + p // 32] = att[0, 4 + p // 32, p % 32]
    attb = attb.astype(bf16)

    x_pad = np.zeros((NPADG, IN_F), np.float32)
    x_pad[:N] = x
    xT = np.ascontiguousarray(x_pad.T).astype(bf16)
    iop = np.tile(np.arange(128, dtype=np.float32)[:, None],
                  (1, 1024)).astype(bf16)
    iotaf = np.tile(np.arange(128, dtype=np.float32), (128, 8)).astype(bf16)
    biasr = np.tile(bias[None, :], (128, 1)).astype(np.float32)
    ident = np.eye(128, dtype=np.float32)

    in_maps = []
    for k in range(CORES):
        xl = np.ascontiguousarray(
            x_pad[k * NPC:k * NPC + NLOC].T).astype(bf16)
        sw = srcc[k].astype(np.int16).reshape(n_super, 64, 16)
        srcw = np.zeros((128, n_super * 64), np.int16)
        srcw[:16, :] = sw.transpose(2, 0, 1).reshape(16, n_super * 64)
        srcw[16:, :] = np.tile(srcw[:16, :], (7, 1))
        in_maps.append({
            "xt": xT, "xtl": xl, "wcat": wcat, "w2t": w2T,
            "attb": attb, "ident": ident.astype(bf16),
            "identf": ident.astype(np.float32), "iop": iop,
            "iotaf": iotaf, "biasr": biasr,
            "qdsb": np.ascontiguousarray(
                qdst[k].reshape(n_tiles, 128).T).astype(bf16),
            "qdrow": qdst[k][None, :].astype(bf16),
            "srcw": srcw,
        })
    return structure, in_maps


def kernel(x, edge_index, W, W1, W2, att, bias):
    global LAST_EXEC_NS
    from concourse import bass_utils

    structure, in_maps = _prep(x, edge_index, W, W1, W2, att, bias)
    if structure not in _CACHE:
        _CACHE[structure] = _build(structure)
    nc = _CACHE[structure]

    trace = bool(int(os.environ.get("GAT_TRACE", "0")))
    res = bass_utils.run_bass_kernel_spmd(
        nc, in_maps, core_ids=list(range(CORES)), trace=trace)
    LAST_EXEC_NS = res.exec_time_ns

    out = np.empty((N, HF), np.float32)
    for k in range(CORES):
        out[k * NPC:(k + 1) * NPC] = res.results[k]["out"][:NPC]
    return out


# revision 32
# speedup vs baseline: 2.1580x; 1.0148x over previous
"""GATv2 layer on 8 Trainium2 NeuronCores (Bass/Tile) — v4.

Strategy: sort edges by dst on host; core k owns dst nodes [2500k, 2500(k+1))
so softmax + aggregation are core-local. Edges grouped per 128-dst block.

Phase overlap: the node table is split in two halves (hsv_a: node tiles
0..78, hsv_b: 79..156, separate DRAM tensors so the tile framework sees
clean dependencies). Each block's edges are partitioned into A-edges
(src in first half) and B-edges; A-supers (8 tiles of 128 edges) only
gather from hsv_a, so their edge processing overlaps the projection of
the B half (emission explicitly interleaves B-projection tiles between
A-supers). Aggregation is two-stage: each block's A-partial (numerator +
softmax denominator) is checkpointed to SBUF (f32) when its A-tiles end,
and added back into the B aggregation PSUM via an identity matmul.

Math notes:
 - logit l_e = att·leaky_relu(hs[src]+hd[dst], 0.2). The ACT engine's Prelu
   (parametric_relu, alpha=0.2) IS leaky relu, so no Relu/linear split.
 - Any per-dst-node additive logit term cancels in the segment softmax,
   so no seg-max subtraction is needed (logits are O(+-5) here).
 - v columns are stored (feature, head)-interleaved so the wv multiply's
   exp broadcast is packed on the last axis (DVE 2x mode); the aggregation
   matmul un-permutes via a strided rhs read.

Per super (1024 edges): one dma_gather (1024 idx, 1KB rows) -> hsv_g
edge-major; qdT = partition_broadcast(dst row) on Pool; ohT/oh one-hots by
is_equal on DVE; per pair of tiles: zT PSUM = PE-transpose(hs) + hd@ohT
(accum groups strictly sequential per 128-col region); lr = Prelu(zT);
logits = lr@attb (edge-major, full PE-array util); exp; wv = v*exp;
segment-sum via oh-matmul accumulated in PSUM per (block, half); block end:
normalize by denominator + bias, DMA out.
"""
import os
import sys

sys.path.insert(0, '/opt/trn_rl_repo')

import numpy as np
import ml_dtypes

N = 20000
IN_F = 128
HEADS = 8
OUT_F = 32
HF = 256          # HEADS * OUT_F
NEG = 0.2
CORES = 8
NPC = 2500        # dst nodes per core
BLOCKS = 20       # 128-node blocks per core
NLOC = BLOCKS * 128
NT_GLOB = 157     # ceil(20000/128)
NPADG = NT_GLOB * 128
NHA_T = 79        # node tiles in half A
NHA = NHA_T * 128             # 10112
NHB = (NT_GLOB - NHA_T) * 128  # 9984

bf16 = ml_dtypes.bfloat16

_CACHE = {}
LAST_EXEC_NS = None


def _build(structure):
    import concourse.bass as bass
    from concourse import mybir, bacc
    from concourse.tile import TileContext

    f32 = mybir.dt.float32
    b16 = mybir.dt.bfloat16
    i16 = mybir.dt.int16
    AF = mybir.ActivationFunctionType
    ALU = mybir.AluOpType

    SA, supers = structure
    n_super = len(supers)
    n_tiles = 8 * n_super

    nc = bacc.Bacc("TRN2", target_bir_lowering=False, debug=False,
                   num_devices=CORES)
    xT = nc.dram_tensor("xt", [128, NPADG], b16, kind="ExternalInput")
    xTl = nc.dram_tensor("xtl", [128, NLOC], b16, kind="ExternalInput")
    wcat = nc.dram_tensor("wcat", [128, 512], b16, kind="ExternalInput")
    w2T = nc.dram_tensor("w2t", [128, 256], b16, kind="ExternalInput")
    attb = nc.dram_tensor("attb", [128, 16], b16, kind="ExternalInput")
    ident = nc.dram_tensor("ident", [128, 128], b16, kind="ExternalInput")
    identf = nc.dram_tensor("identf", [128, 128], f32, kind="ExternalInput")
    iop = nc.dram_tensor("iop", [128, 1024], b16, kind="ExternalInput")
    iotaf = nc.dram_tensor("iotaf", [128, 1024], b16, kind="ExternalInput")
    qdsb = nc.dram_tensor("qdsb", [128, n_tiles], b16, kind="ExternalInput")
    qdrow = nc.dram_tensor("qdrow", [1, n_tiles * 128], b16,
                           kind="ExternalInput")
    srcw = nc.dram_tensor("srcw", [128, n_super * 64], i16,
                          kind="ExternalInput")
    biasr = nc.dram_tensor("biasr", [128, 256], f32, kind="ExternalInput")
    outt = nc.dram_tensor("out", [NLOC, 256], f32, kind="ExternalOutput")

    hsv_a = nc.dram_tensor("hsva", [NHA, 512], b16, kind="Internal")
    hsv_b = nc.dram_tensor("hsvb", [NHB, 512], b16, kind="Internal")

    with TileContext(nc) as tc:
        with tc.tile_pool(name="const", bufs=1) as cp:
            wcat_sb = cp.tile([128, 512], b16)
            nc.sync.dma_start(wcat_sb[:], wcat[:])
            w2_sb = cp.tile([128, 256], b16)
            nc.sync.dma_start(w2_sb[:], w2T[:])
            attb_sb = cp.tile([128, 16], b16)
            nc.sync.dma_start(attb_sb[:], attb[:])
            id_sb = cp.tile([128, 128], b16)
            nc.sync.dma_start(id_sb[:], ident[:])
            idf_sb = cp.tile([128, 128], f32)
            nc.sync.dma_start(idf_sb[:], identf[:])
            iop_sb = cp.tile([128, 1024], b16)
            nc.sync.dma_start(iop_sb[:], iop[:])
            iof_sb = cp.tile([128, 1024], b16)
            nc.sync.dma_start(iof_sb[:], iotaf[:])
            qd_sb = cp.tile([128, n_tiles], b16)
            nc.sync.dma_start(qd_sb[:], qdsb[:])
            src_sb = cp.tile([128, n_super * 64], i16)
            nc.sync.dma_start(src_sb[:], srcw[:])
            bias_sb = cp.tile([128, 256], f32)
            nc.sync.dma_start(bias_sb[:], biasr[:])
            stA = []
            for b in range(BLOCKS):
                _t = cp.tile([128, 264], f32, tag=f"stA{b}",
                             name=f"stA{b}")
                stA.append(_t)
            hd_tiles = [None] * BLOCKS

            NJC = 20    # node tiles per x chunk
            NST = 8     # node tiles per staging/write batch

            # ---------------- phase 1a: half-A projections + hd ----------
            with tc.tile_pool(name="proj", bufs=2) as pp, \
                 tc.tile_pool(name="pst", bufs=3) as pstp, \
                 tc.tile_pool(name="pps", bufs=6, space="PSUM") as pps, \
                 tc.tile_pool(name="pps2", bufs=2, space="PSUM") as pps2:
                xl = pp.tile([128, NLOC], b16, tag="xl")
                nc.sync.dma_start(xl[:], xTl[:])
                cast_rr = 0
                stage = None
                for j in range(NHA_T):
                    if j % NJC == 0:
                        xc = pp.tile([128, 2560], b16, tag="xc")
                        hi = min((j + NJC) * 128, NHA)
                        nc.sync.dma_start(xc[:, 0:hi - j * 128],
                                          xT[:, j * 128:hi])
                    if j % NST == 0:
                        stage = pstp.tile([128, NST * 512], b16, tag="st")
                    xcol = (j % NJC) * 128
                    ps = pps.tile([128, 512], f32)
                    nc.tensor.matmul(ps[:], lhsT=xc[:, xcol:xcol + 128],
                                     rhs=wcat_sb[:], start=True, stop=True)
                    dstsl = stage[:, (j % NST) * 512:(j % NST) * 512 + 512]
                    if cast_rr == 0:
                        nc.scalar.copy(dstsl, ps[:])
                    else:
                        nc.vector.tensor_copy(dstsl, ps[:])
                    cast_rr = (cast_rr + 1) % 2
                    if j % NST == NST - 1 or j == NHA_T - 1:
                        j0 = (j // NST) * NST
                        nt = j - j0 + 1
                        nc.sync.dma_start(
                            hsv_a[j0 * 128:(j0 + nt) * 128, :].rearrange(
                                "(t p) f -> p t f", p=128),
                            stage[:, 0:nt * 512].rearrange(
                                "p (t f) -> p t f", f=512))
                    # one local-dst (hd) tile every 4 node tiles
                    if j % 4 == 3 and j // 4 < BLOCKS:
                        b = j // 4
                        ps2 = pps2.tile([128, 256], f32)
                        nc.tensor.matmul(ps2[:],
                                         lhsT=xl[:, b * 128:b * 128 + 128],
                                         rhs=w2_sb[:], start=True, stop=True)
                        hdt = cp.tile([128, 256], b16, tag=f"hd{b}")
                        if b % 2 == 0:
                            nc.scalar.copy(hdt[:], ps2[:])
                        else:
                            nc.vector.tensor_copy(hdt[:], ps2[:])
                        hd_tiles[b] = hdt
                for b in range(BLOCKS):
                    if hd_tiles[b] is not None:
                        continue
                    ps2 = pps2.tile([128, 256], f32)
                    nc.tensor.matmul(ps2[:],
                                     lhsT=xl[:, b * 128:b * 128 + 128],
                                     rhs=w2_sb[:], start=True, stop=True)
                    hdt = cp.tile([128, 256], b16, tag=f"hd{b}")
                    nc.vector.tensor_copy(hdt[:], ps2[:])
                    hd_tiles[b] = hdt

            # ------- phase 1b (half-B projections) + phase 2, interleaved --
            with tc.tile_pool(name="projB", bufs=2) as ppB, \
                 tc.tile_pool(name="pstB", bufs=2) as pstB, \
                 tc.tile_pool(name="ppsB", bufs=2, space="PSUM") as ppsB, \
                 tc.tile_pool(name="edge", bufs=5) as ep, \
                 tc.tile_pool(name="edgeq", bufs=3) as epq, \
                 tc.tile_pool(name="pair", bufs=6) as rp, \
                 tc.tile_pool(name="zps", bufs=2, space="PSUM") as zps, \
                 tc.tile_pool(name="lps", bufs=2, space="PSUM") as lps, \
                 tc.tile_pool(name="aps", bufs=1, space="PSUM") as aps, \
                 tc.tile_pool(name="apd", bufs=1, space="PSUM") as apd, \
                 tc.tile_pool(name="np_", bufs=2) as npl:

                bstate = {"xc": None, "stage": None, "cast_rr": 0}

                def emit_b_tile(jb):
                    # node tile NHA_T + jb -> hsv_b rows jb*128..
                    j = NHA_T + jb
                    if jb % NJC == 0:
                        bstate["xc"] = ppB.tile([128, 2560], b16, tag="xcB", name="xcB")
                        hi = min((jb + NJC) * 128 + NHA, NPADG)
                        nc.sync.dma_start(
                            bstate["xc"][:, 0:hi - j * 128],
                            xT[:, j * 128:hi])
                    if jb % NST == 0:
                        bstate["stage"] = pstB.tile([128, NST * 512], b16,
                                                    tag="stB", name="stB")
                    xcol = (jb % NJC) * 128
                    ps = ppsB.tile([128, 512], f32)
                    nc.tensor.matmul(ps[:],
                                     lhsT=bstate["xc"][:, xcol:xcol + 128],
                                     rhs=wcat_sb[:], start=True, stop=True)
                    dstsl = bstate["stage"][:, (jb % NST) * 512:
                                            (jb % NST) * 512 + 512]
                    if bstate["cast_rr"] == 0:
                        nc.scalar.copy(dstsl, ps[:])
                    else:
                        nc.vector.tensor_copy(dstsl, ps[:])
                    bstate["cast_rr"] = (bstate["cast_rr"] + 1) % 2
                    nb_t = NT_GLOB - NHA_T
                    if jb % NST == NST - 1 or jb == nb_t - 1:
                        j0 = (jb // NST) * NST
                        nt = jb - j0 + 1
                        nc.sync.dma_start(
                            hsv_b[j0 * 128:(j0 + nt) * 128, :].rearrange(
                                "(t p) f -> p t f", p=128),
                            bstate["stage"][:, 0:nt * 512].rearrange(
                                "p (t f) -> p t f", f=512))

                agst = {"aggp": None, "aggd": None}

                def normalize_out(b):
                    dn = npl.tile([128, 8], f32, tag="dn")
                    nc.vector.tensor_scalar(
                        out=dn[:], in0=agst["aggd"][:],
                        scalar1=1e-12, scalar2=None, op0=ALU.max)
                    rec = npl.tile([128, 8], f32, tag="rec")
                    nc.vector.reciprocal(rec[:], dn[:])
                    osb = npl.tile([128, 256], f32, tag="osb")
                    nc.vector.tensor_tensor(
                        out=osb[:].rearrange("p (h f) -> p h f", h=8),
                        in0=agst["aggp"][:].rearrange("p (h f) -> p h f", h=8),
                        in1=rec[:].unsqueeze(2).broadcast_to([128, 8, 32]),
                        op=ALU.mult)
                    nc.vector.tensor_add(osb[:], osb[:], bias_sb[:])
                    nc.sync.dma_start(outt[b * 128:(b + 1) * 128, :], osb[:])

                def process_super(s, sup):
                    table = hsv_a if s < SA else hsv_b
                    idsl = src_sb[:, s * 64:(s + 1) * 64]
                    hsv_g = ep.tile([128, 8 * 512], b16, tag="hsvg")
                    nc.gpsimd.dma_gather(
                        hsv_g[:].rearrange("p (t f) -> p t f", f=512),
                        table[:], idsl, 1024, 1024, 512,
                        transpose=False)
                    qstage = epq.tile([128, 1024], b16, tag="qst")
                    nc.sync.dma_start(qstage[0:1, :],
                                      qdrow[:, s * 1024:(s + 1) * 1024])
                    qdT = epq.tile([128, 1024], b16, tag="qdT")
                    nc.gpsimd.partition_broadcast(qdT[:], qstage[0:1, :])
                    ohT = ep.tile([128, 1024], b16, tag="ohT")
                    nc.vector.tensor_tensor(out=ohT[:], in0=qdT[:],
                                            in1=iop_sb[:], op=ALU.is_equal)
                    oh = ep.tile([128, 1024], b16, tag="oh")
                    nc.vector.tensor_tensor(
                        out=oh[:].rearrange("p (t c) -> p t c", t=8),
                        in0=qd_sb[:, s * 8:(s + 1) * 8].unsqueeze(2)
                            .broadcast_to([128, 8, 128]),
                        in1=iof_sb[:].rearrange("p (t c) -> p t c", t=8),
                        op=ALU.is_equal)
                    tof = 0
                    for ui, unit in enumerate(sup):
                        blk, dead, ntl, tflags = unit
                        hdt = hd_tiles[blk]
                        u0 = tof
                        tof += ntl
                        W = 256 * ntl
                        # zT in PSUM; per 128-col region transpose(hs) opens
                        # the accum group, hd one-hot closes it
                        psZ = zps.tile([128, 512], f32, tag="z")
                        for hf in range(2):
                            for sl in range(ntl):
                                reg = psZ[:, hf * 128 * ntl + sl * 128:
                                          hf * 128 * ntl + sl * 128 + 128]
                                nc.tensor.matmul(
                                    reg,
                                    lhsT=hsv_g[:, (u0 + sl) * 512 +
                                               hf * 128:
                                               (u0 + sl) * 512 +
                                               hf * 128 + 128],
                                    rhs=id_sb[:], start=True, stop=False)
                                nc.tensor.matmul(
                                    reg,
                                    lhsT=hdt[:, hf * 128:hf * 128 + 128],
                                    rhs=ohT[:, (u0 + sl) * 128:
                                            (u0 + sl) * 128 + 128],
                                    start=False, stop=True)
                        lr = rp.tile([128, 512], b16, tag="lr")
                        nc.scalar.activation(lr[:, 0:W], psZ[:, 0:W],
                                             AF.Prelu, alpha=NEG)
                        lgt = lps.tile([128, 16], f32)
                        for sl in range(ntl):
                            for hf in range(2):
                                nc.tensor.matmul(
                                    lgt[:, sl * 8:sl * 8 + 8],
                                    lhsT=lr[:, hf * 128 * ntl + sl * 128:
                                            hf * 128 * ntl + sl * 128 + 128],
                                    rhs=attb_sb[:, hf * 8:hf * 8 + 8],
                                    start=(hf == 0), stop=(hf == 1))
                        wvp = rp.tile([128, 528], b16, tag="wvp")
                        nc.scalar.activation(
                            wvp[:].rearrange("p (t c) -> p t c", t=2)
                                [:, 0:ntl, 256:264],
                            lgt[:, 0:8 * ntl].rearrange(
                                "p (a b) -> p a b", b=8),
                            AF.Exp)
                        nc.vector.tensor_tensor(
                            out=wvp[:].rearrange("p (t c) -> p t c", t=2)
                                [:, 0:ntl, 0:256].rearrange(
                                    "p t (f h) -> p t f h", h=8),
                            in0=hsv_g[:].rearrange("p (t f) -> p t f", f=512)
                                [:, u0:u0 + ntl, 256:512].rearrange(
                                    "p t (f h) -> p t f h", h=8),
                            in1=wvp[:].rearrange("p (t c) -> p t c", t=2)
                                [:, 0:ntl, 256:264].unsqueeze(2)
                                .broadcast_to([128, ntl, 32, 8]),
                            op=ALU.mult)
                        if dead:
                            continue
                        for sl in range(ntl):
                            tstart, tstop = tflags[sl]
                            ohsl = oh[:, (u0 + sl) * 128:
                                      (u0 + sl) * 128 + 128]
                            if tstart:
                                agst["aggp"] = aps.tile([128, 256], f32,
                                                        tag="agg", name="agg")
                                agst["aggd"] = apd.tile([128, 8], f32,
                                                        tag="aggd", name="aggd")
                            hardstop = tstop in (1, 3)
                            nc.tensor.matmul(
                                agst["aggp"][:],
                                lhsT=ohsl,
                                rhs=wvp[:, sl * 264:sl * 264 + 256]
                                    .rearrange("p (f h) -> p h f", h=8),
                                start=tstart, stop=hardstop)
                            nc.tensor.matmul(
                                agst["aggd"][:],
                                lhsT=ohsl,
                                rhs=wvp[:, sl * 264 + 256:sl * 264 + 264],
                                start=tstart, stop=hardstop)
                            if tstop == 1:
                                # close A-partial: checkpoint to SBUF (f32)
                                nc.scalar.copy(stA[blk][:, 0:256],
                                               agst["aggp"][:])
                                nc.vector.tensor_copy(stA[blk][:, 256:264],
                                                      agst["aggd"][:])
                            elif tstop == 2:
                                # combine A-partial back, then finalize
                                nc.tensor.matmul(
                                    agst["aggp"][:], lhsT=idf_sb[:],
                                    rhs=stA[blk][:, 0:256],
                                    start=False, stop=True)
                                nc.tensor.matmul(
                                    agst["aggd"][:], lhsT=idf_sb[:],
                                    rhs=stA[blk][:, 256:264],
                                    start=False, stop=True)
                                normalize_out(blk)
                            elif tstop == 3:
                                normalize_out(blk)

                nb_t = NT_GLOB - NHA_T
                done_b = 0
                for s in range(SA):
                    share = ((s + 1) * nb_t + SA - 1) // SA
                    while done_b < min(share, nb_t):
                        emit_b_tile(done_b)
                        done_b += 1
                    process_super(s, supers[s])
                while done_b < nb_t:
                    emit_b_tile(done_b)
                    done_b += 1
                for s in range(SA, n_super):
                    process_super(s, supers[s])
    nc.compile()
    return nc


def _prep(x, edge_index, W, W1, W2, att, bias):
    x = np.asarray(x, np.float32)
    ei = np.asarray(edge_index)
    W = np.asarray(W, np.float32)
    W1 = np.asarray(W1, np.float32)
    W2 = np.asarray(W2, np.float32)
    att = np.asarray(att, np.float32)
    bias = np.asarray(bias, np.float32)

    src = ei[0].astype(np.int64)
    dst = ei[1].astype(np.int64)
    perm = np.argsort(dst, kind='stable')
    src_s = src[perm].astype(np.int32)
    dst_s = dst[perm].astype(np.int32)

    core_of = dst_s // NPC
    lblk = (dst_s - core_of * NPC) // 128
    cnt = np.zeros((CORES, BLOCKS), np.int64)
    np.add.at(cnt, (core_of, lblk), 1)

    # per-core tile streams; identical structure required across cores for
    # one SPMD program -> build per-core structures and take the max layout
    per_core = []
    for k in range(CORES):
        tiles = {"A": [], "B": []}   # (blk, src128, qd128, first, last)
        for b in range(BLOCKS):
            c = cnt[k, b]
            lo = np.searchsorted(dst_s, k * NPC + b * 128)
            e_src = src_s[lo:lo + c]
            e_qd = (dst_s[lo:lo + c] - k * NPC - b * 128).astype(np.float32)
            isA = e_src < NHA
            for side in ("A", "B"):
                m = isA if side == "A" else ~isA
                s_arr = e_src[m]
                q_arr = e_qd[m]
                if side == "B":
                    s_arr = s_arr - NHA
                ne = len(s_arr)
                if ne == 0:
                    continue
                T = (ne + 127) // 128
                sp = np.zeros(T * 128, np.int32)
                qp = np.full(T * 128, -1.0, np.float32)
                sp[:ne] = s_arr
                qp[:ne] = q_arr
                for i in range(T):
                    tiles[side].append(
                        (b, sp[i * 128:(i + 1) * 128],
                         qp[i * 128:(i + 1) * 128], i == 0, i == T - 1))
        per_core.append(tiles)

    # SPMD: all cores share one program; pad every core to the max tile
    # count per side and union the stop/start structure. Simplest correct
    # approach: use core-0's structure only if all cores match; otherwise
    # pad each (blk, side) run to the max T across cores.
    # Rebuild with uniform per-(blk, side) tile counts:
    Tmax = {}
    for side in ("A", "B"):
        for b in range(BLOCKS):
            m = 0
            for k in range(CORES):
                tl = [t for t in per_core[k][side] if t[0] == b]
                m = max(m, len(tl))
            Tmax[(side, b)] = m

    supers_struct = []
    tile_arrays = []   # per core appended later
    order = []         # (side, blk, idx_in_run, first, last)
    for side in ("A", "B"):
        for b in range(BLOCKS):
            T = Tmax[(side, b)]
            for i in range(T):
                order.append((side, b, i, i == 0, i == T - 1))
        # pad to super boundary with dead tiles
        while len(order) % 8:
            order.append((side, 0, -1, False, False))
        if side == "A":
            nA_tiles = len(order)
    SA = nA_tiles // 8
    n_tiles = len(order)
    n_super = n_tiles // 8

    # stop codes: per (side, blk): last tile -> 1 (partial) if the other
    # side has tiles for this blk, else 3; side B last -> 2 if A had tiles
    def tile_flags(to):
        side, b, i, first, last = to
        stop = 0
        if last:
            if side == "A":
                stop = 1 if Tmax[("B", b)] > 0 else 3
            else:
                stop = 2 if Tmax[("A", b)] > 0 else 3
        return (bool(first), int(stop))

    for s in range(n_super):
        sup = []
        t = 0
        while t < 8:
            t0o = order[s * 8 + t]
            side0, b0, i0 = t0o[0], t0o[1], t0o[2]
            pair = False
            if t < 7:
                t1o = order[s * 8 + t + 1]
                if (i0 >= 0 and t1o[0] == side0 and t1o[1] == b0
                        and t1o[2] == i0 + 1 and i0 % 2 == 0):
                    pair = True
                elif i0 < 0 and t1o[2] < 0:
                    pair = True
            dead = i0 < 0
            if pair:
                sup.append((int(b0), bool(dead), 2,
                            (tile_flags(t0o), tile_flags(t1o))))
                t += 2
            else:
                sup.append((int(b0), bool(dead), 1,
                            (tile_flags(t0o),)))
                t += 1
        supers_struct.append(tuple(sup))
    structure = (SA, tuple(supers_struct))

    # per-core edge arrays following `order`
    srcc = np.zeros((CORES, n_tiles * 128), np.int32)
    qdst = np.full((CORES, n_tiles * 128), -1.0, np.float32)
    for k in range(CORES):
        runs = {}
        for side in ("A", "B"):
            for t in per_core[k][side]:
                runs.setdefault((side, t[0]), []).append(t)
        for ti, (side, b, i, first, last) in enumerate(order):
            if i < 0:
                continue
            run = runs.get((side, b), [])
            if i < len(run):
                srcc[k, ti * 128:(ti + 1) * 128] = run[i][1]
                qdst[k, ti * 128:(ti + 1) * 128] = run[i][2]

    # weights, bf16: wcat = [W1.T | W.T-(f,h)-interleaved], w2T
    vperm = (np.arange(8)[None, :] * 32
             + np.arange(32)[:, None]).reshape(256)
    wcat = np.concatenate([W1.T, W.T[:, vperm]], axis=1).astype(bf16)
    w2T = np.ascontiguousarray(W2.T).astype(bf16)

    # att blocks (unscaled; Prelu handles the leaky slope exactly)
    attb = np.zeros((128, 16), np.float32)
    for p in range(128):
        attb[p, p // 32] = att[0, p // 32, p % 32]
        attb[p, 8 + 4 + p // 32] = att[0, 4 + p // 32, p % 32]
    attb = attb.astype(bf16)

    x_pad = np.zeros((NPADG, IN_F), np.float32)
    x_pad[:N] = x
    xT = np.ascontiguousarray(x_pad.T).astype(bf16)
    iop = np.tile(np.arange(128, dtype=np.float32)[:, None],
                  (1, 1024)).astype(bf16)
    iotaf = np.tile(np.arange(128, dtype=np.float32), (128, 8)).astype(bf16)
    biasr = np.tile(bias[None, :], (128, 1)).astype(np.float32)
    ident = np.eye(128, dtype=np.float32)

    in_maps = []
    for k in range(CORES):
        xl = np.ascontiguousarray(
            x_pad[k * NPC:k * NPC + NLOC].T).astype(bf16)
        sw = srcc[k].astype(np.int16).reshape(n_super, 64, 16)
        srcw = np.zeros((128, n_super * 64), np.int16)
        srcw[:16, :] = sw.transpose(2, 0, 1).reshape(16, n_super * 64)
        srcw[16:, :] = np.tile(srcw[:16, :], (7, 1))
        in_maps.append({
            "xt": xT, "xtl": xl, "wcat": wcat, "w2t": w2T,
            "attb": attb, "ident": ident.astype(bf16),
            "identf": ident.astype(np.float32), "iop": iop,
            "iotaf": iotaf, "biasr": biasr,
            "qdsb": np.ascontiguousarray(
                qdst[k].reshape(n_tiles, 128).T).astype(bf16),
            "qdrow": qdst[k][None, :].astype(bf16),
            "srcw": srcw,
        })
    return structure, in_maps


def kernel(x, edge_index, W, W1, W2, att, bias):
    global LAST_EXEC_NS
    from concourse import bass_utils

    structure, in_maps = _prep(x, edge_index, W, W1, W2, att, bias)
    if structure not in _CACHE:
        _CACHE[structure] = _build(structure)
    nc = _CACHE[structure]

    trace = bool(int(os.environ.get("GAT_TRACE", "0")))
    res = bass_utils.run_bass_kernel_spmd(
        nc, in_maps, core_ids=list(range(CORES)), trace=trace)
    LAST_EXEC_NS = res.exec_time_ns

    out = np.empty((N, HF), np.float32)
    for k in range(CORES):
        out[k * NPC:(k + 1) * NPC] = res.results[k]["out"][:NPC]
    return out
